# revision 1
# baseline (speedup 1.0000x reference)
"""DeeperGCN (GENConv softmax-aggr, L=2) Trainium2 kernel, 8-core SPMD.

Strategy:
  - Nodes 1D-partitioned: core k owns 6250 nodes (padded to 6272 = 49*128).
  - Per layer, each core computes node-level message tables
    E = exp(t*m), Wt = m*E with m = relu(h)+eps for its shard (fp16),
    AllGathers the full [50176, 256] table, then processes its incident
    edges (grouped by dst block of 128 nodes) with:
      indirect-DMA row gather  ->  one-hot matmul scatter-accumulate in PSUM.
    softmax aggregate = wsum/ssum computed as exp(ln(wsum)-ln(ssum)).
  - MLP: w1/w2 kept stationary on PE; BatchNorm stats via bn_stats/bn_aggr
    (equal 250-wide tiles, exact combine) + AllReduce; LayerNorm per node
    after PE transpose back to node-major.
  - Dtypes: fp32 everywhere except the gathered message tables / one-hot
    matrices (fp16) whose products accumulate in fp32 PSUM.
"""

import os
import sys
import math

import numpy as np

sys.path.insert(0, "/opt/trn_rl_repo")

# Problem constants (hardcoded per contract)
N = 50000
E_EDGES = 640000
D = 128
D2 = 256
L = 2
C_IN = 128
C_OUT = 64
MSG_EPS = 1e-7
W = 8           # cores
P = 128         # partitions
SH_REAL = N // W          # 6250 real nodes per core
NB = math.ceil(SH_REAL / P)   # 49 node blocks per core
SH = NB * P               # 6272 padded nodes per core
NPAD = SH * W             # 50176
BT = 250                  # BatchNorm stats tile width (SH_REAL % BT == 0)
MT = 512                  # MLP node-tile width


def default_params():
    return dict(
        W=W, P=P, D=D, D2=D2, L=L, C_OUT=C_OUT, SH=SH, SH_REAL=SH_REAL,
        NB=NB, NPAD=NPAD, BT=BT, MT=MT, MSG_EPS=MSG_EPS,
        CAPL=10, CAPH=6, LOSPLIT=32768, GRP=2,
        # fast-path flags (host-verified against actual input values)
        t_one=True, in_b_zero=True, out_b_zero=True, ln_identity=True,
        b2_zero=True,
    )


def build_program(p):
    from concourse import bacc, bass, mybir, tile
    from concourse.bass import IndirectOffsetOnAxis
    from concourse.masks import make_identity
    from contextlib import ExitStack

    dt = mybir.dt
    f32, f16, i32 = dt.float32, dt.float16, dt.int32
    AF = mybir.ActivationFunctionType
    OP = mybir.AluOpType

    Wn, Pn, Dn, D2n = p["W"], p["P"], p["D"], p["D2"]
    Ln, COUT = p["L"], p["C_OUT"]
    SHn, SHR, NBn, NPADn = p["SH"], p["SH_REAL"], p["NB"], p["NPAD"]
    BTn, MTn = p["BT"], p["MT"]
    CAPL, CAPH, LOSPLIT, GRP = p["CAPL"], p["CAPH"], p["LOSPLIT"], p["GRP"]
    CAP = CAPL + CAPH
    NG = math.ceil(NBn / GRP)     # gather groups
    NBT = SHR // BTn              # bn stats tiles
    NMT = math.ceil(SHn / MTn)    # mlp node tiles
    eps_msg = p["MSG_EPS"]

    nc = bacc.Bacc(
        "TRN2", target_bir_lowering=False, debug=False,
        enable_asserts=False, num_devices=Wn, num_swdge_queues=4,
    )

    def din(name, shape, dty):
        return nc.dram_tensor(name, shape, dty, kind="ExternalInput").ap()

    i16 = dt.int16
    x_fm_d = din("x_fm", [Dn, SHn], f32)            # host-transposed x shard
    idx16_d = din("idx16", [NG, Pn, GRP * CAP * 8], i16)  # gather indices
    dst_col_d = din("dst_col", [NBn, Pn, CAP], f32)  # dst one-hot columns
    in_w_d = din("in_w", [Dn, Dn], f32)
    w1_d = din("w1", [Ln, Dn, D2n], f32)
    w2_d = din("w2", [Ln, D2n, Dn], f32)
    bn_g_d = din("bn_g", [Ln, D2n], f32)
    bn_b_d = din("bn_b", [Ln, D2n], f32)
    out_w_d = din("out_w", [Dn, COUT], f32)
    if not p["b2_zero"]:
        b2_d = din("b2", [Ln, Dn], f32)
    if not p["t_one"]:
        t_d = din("t", [Ln], f32)
    if not p["in_b_zero"]:
        in_b_d = din("in_b", [Dn], f32)
    if not p["out_b_zero"]:
        out_b_d = din("out_b", [COUT], f32)
    if not p["ln_identity"]:
        ln_g_d = din("ln_g", [Ln, Dn], f32)
        ln_b_d = din("ln_b", [Ln, Dn], f32)

    out_d = nc.dram_tensor("out", [SHn, COUT], f32, kind="ExternalOutput").ap()

    rg = [list(range(Wn))]

    with ExitStack() as ctx:
        tc = ctx.enter_context(tile.TileContext(nc))
        sb = ctx.enter_context(tc.tile_pool(name="sb", bufs=1))
        sb2 = ctx.enter_context(tc.tile_pool(name="sb2", bufs=2))
        pp = ctx.enter_context(tc.tile_pool(name="pp", bufs=2, space="PSUM"))
        dr = ctx.enter_context(tc.tile_pool(name="dr", bufs=2, space="DRAM"))

        # ---- constants / weights resident in SBUF ----
        ident = sb.tile([Pn, Pn], f32, tag="ident")
        make_identity(nc, ident[:])
        iota_sb = sb.tile([Pn, Pn], f16, tag="iota")
        nc.gpsimd.iota(iota_sb[:], pattern=[[1, Pn]], base=0,
                       channel_multiplier=0,
                       allow_small_or_imprecise_dtypes=True)

        in_w_sb = sb.tile([Pn, Dn], f32, tag="in_w")
        nc.sync.dma_start(out=in_w_sb[:], in_=in_w_d)
        w1_sb = sb.tile([Pn, Ln, D2n], f32, tag="w1")
        w2_sb = sb.tile([Pn, Ln, 2, Dn], f32, tag="w2")
        bng_sb = sb.tile([Pn, Ln, 2], f32, tag="bng")
        bnb_sb = sb.tile([Pn, Ln, 2], f32, tag="bnb")
        for l in range(Ln):
            nc.sync.dma_start(out=w1_sb[:, l, :], in_=w1_d[l])
            for ch in range(2):
                nc.sync.dma_start(out=w2_sb[:, l, ch, :],
                                  in_=w2_d[l, ch * Pn:(ch + 1) * Pn, :])
            nc.sync.dma_start(
                out=bng_sb[:, l, :],
                in_=bn_g_d[l].rearrange("(c p) -> p c", p=Pn))
            nc.sync.dma_start(
                out=bnb_sb[:, l, :],
                in_=bn_b_d[l].rearrange("(c p) -> p c", p=Pn))
        out_w_sb = sb.tile([Pn, COUT], f32, tag="out_w")
        nc.sync.dma_start(out=out_w_sb[:], in_=out_w_d)

        ones_row = sb.tile([1, Pn], f32, tag="ones_row")
        nc.vector.memset(ones_row[:], 1.0)

        def const_col(val, tagname):
            tcol = sb.tile([Pn, 1], f32, tag=tagname)
            nc.vector.memset(tcol[:], val)
            return tcol

        c_1e16 = const_col(1e-16, "c_1e16")
        c_1e30 = const_col(1e-30, "c_1e30")
        c_1e5 = const_col(1e-5, "c_1e5")

        def bcast_row(dram_row_ap, width, tagname):
            """[1,width] dram -> [128,width] sbuf via ones-matmul."""
            row = sb.tile([1, width], f32, tag=tagname + "_r")
            nc.sync.dma_start(out=row[:], in_=dram_row_ap)
            ps = pp.tile([Pn, width], f32, tag="psm", name=tagname + "_ps")
            nc.tensor.matmul(ps[:], lhsT=ones_row[:], rhs=row[:],
                             start=True, stop=True)
            out = sb.tile([Pn, width], f32, tag=tagname)
            nc.scalar.activation(out[:], ps[:], AF.Copy)
            return out

        if not p["b2_zero"]:
            b2c_sb = sb.tile([Pn, Ln], f32, tag="b2c")
            for l in range(Ln):
                nc.sync.dma_start(out=b2c_sb[:, l:l + 1], in_=b2_d[l][:, None])
        if not p["t_one"]:
            t_bc = bcast_row(t_d[None, :], Ln, "t_bc")  # [128, L]
        if not p["in_b_zero"]:
            inb_bc = bcast_row(in_b_d[None, :], Dn, "inb_bc")
        if not p["out_b_zero"]:
            outb_bc = bcast_row(out_b_d[None, :], COUT, "outb_bc")
        if not p["ln_identity"]:
            lng_bc = [bcast_row(ln_g_d[l][None, :], Dn, f"lng{l}")
                      for l in range(Ln)]
            lnb_bc = [bcast_row(ln_b_d[l][None, :], Dn, f"lnb{l}")
                      for l in range(Ln)]

        # ---- edge metadata ----
        gsems = [nc.alloc_semaphore(f"gsem{q}") for q in range(4)]
        prep_counter = [0]

        def next_q():
            q = prep_counter[0] % 4
            prep_counter[0] += 1
            return q
        dstc_sb = sb.tile([Pn, NBn, CAP], f32, tag="dstc")
        nc.sync.dma_start(out=dstc_sb[:],
                          in_=dst_col_d.rearrange("b p c -> p b c"))

        # ---- persistent state ----
        h_sb = sb.tile([Pn, NBn, Dn], f32, tag="h")      # node-major h shard
        X_fm = sb.tile([Pn, SHn], f32, tag="Xfm")        # feature-major agg+h
        h2T_sb = sb.tile([Pn, NBn, Dn], f32, tag="h2T")  # node-major h2

        # ---- in-projection: h0 = x @ in_w (+ in_b) ----
        # X_fm doubles as the staging buffer for the transposed x shard;
        # the layer-0 edge phase overwrites it only after in-proj reads it.
        nc.sync.dma_start(out=X_fm[:], in_=x_fm_d)
        for b in range(NBn):
            h0_ps = pp.tile([Pn, Dn], f32, tag="psm", name="h0_ps")
            nc.tensor.matmul(h0_ps[:], lhsT=X_fm[:, b * Pn:(b + 1) * Pn],
                             rhs=in_w_sb[:], start=True, stop=True)
            nc.scalar.activation(h_sb[:, b, :], h0_ps[:], AF.Copy)
            if not p["in_b_zero"]:
                nc.vector.tensor_add(h_sb[:, b, :], h_sb[:, b, :], inb_bc[:])

        # ---- layers ----
        for l in range(Ln):
            # -- node-level message tables: ew = [exp(t*m) | m*exp(t*m)] fp16
            ew_sb = sb.tile([Pn, NBn, 2 * Dn], f16, tag="ew", name="ew_sb")
            for b in range(NBn):
                m_sb = sb2.tile([Pn, Dn], f32, tag="m", name="m_sb")
                nc.vector.tensor_scalar(
                    out=m_sb[:], in0=h_sb[:, b, :], scalar1=0.0,
                    scalar2=eps_msg, op0=OP.max, op1=OP.add)
                e_sb = sb2.tile([Pn, Dn], f32, tag="e", name="e_sb")
                if p["t_one"]:
                    nc.scalar.activation(e_sb[:], m_sb[:], AF.Exp)
                else:
                    nc.scalar.activation(e_sb[:], m_sb[:], AF.Exp,
                                         scale=t_bc[:, l:l + 1])
                nc.vector.tensor_copy(ew_sb[:, b, 0:Dn], e_sb[:])
                nc.vector.tensor_mul(ew_sb[:, b, Dn:2 * Dn], m_sb[:], e_sb[:])

            ew_shard = dr.tile([SHn, 2 * Dn], f16, tag="ew_shard",
                               name="ew_shard")
            nc.sync.dma_start(
                out=ew_shard[:].rearrange("(b p) f -> p b f", p=Pn),
                in_=ew_sb[:])
            ew_full = dr.tile([NPADn, 2 * Dn], f16, tag="ew_full",
                              addr_space="Shared", name="ew_full")
            nc.gpsimd.collective_compute(
                "AllGather", OP.bypass, replica_groups=rg,
                ins=[ew_shard[:]], outs=[ew_full[:]])

            # -- edge aggregation: per group of GRP dst blocks, two
            #    dma_gathers (lo table rows [0,LOSPLIT), hi from LOSPLIT),
            #    then per block one-hot matmul accumulation --
            for g in range(NG):
                nblk = min(GRP, NBn - g * GRP)   # real blocks in group
                idxt = sb2.tile([Pn, GRP * CAP * 8], i16, tag="idxt",
                                name="idxt")
                nc.sync.dma_start(out=idxt[:], in_=idx16_d[g])
                GW = sb2.tile([Pn, GRP * CAP, 2 * Dn], f16, tag="gw",
                              name="GW")
                nlo = GRP * CAPL * Pn
                nc.gpsimd.dma_gather(
                    out_ap=GW[:, 0:GRP * CAPL, :], in_ap=ew_full[:],
                    idxs_ap=idxt[:, 0:GRP * CAPL * 8],
                    num_idxs=nlo, num_idxs_reg=nlo, elem_size=2 * Dn,
                    single_packet=False)
                if CAPH > 0:
                    nhi = GRP * CAPH * Pn
                    nc.gpsimd.dma_gather(
                        out_ap=GW[:, GRP * CAPL:GRP * CAP, :],
                        in_ap=ew_full[LOSPLIT:, :],
                        idxs_ap=idxt[:, GRP * CAPL * 8:GRP * CAP * 8],
                        num_idxs=nhi, num_idxs_reg=nhi, elem_size=2 * Dn,
                        single_packet=False)
                for s_ in range(nblk):
                    b = g * GRP + s_
                    S = sb2.tile([Pn, CAP, Dn], f16, tag="S", name="S")
                    for c in range(CAP):
                        eng = nc.vector if c % 2 == 0 else nc.gpsimd
                        eng.tensor_scalar(
                            out=S[:, c, :], in0=iota_sb[:],
                            scalar1=dstc_sb[:, b, c:c + 1], scalar2=None,
                            op0=OP.is_equal)
                    pblk = pp.tile([Pn, 2 * Dn], f32, tag="pblk", name="pblk")
                    for c in range(CAP):
                        gc = (s_ * CAPL + c) if c < CAPL else (
                            GRP * CAPL + s_ * CAPH + (c - CAPL))
                        nc.tensor.matmul(pblk[:], lhsT=S[:, c, :],
                                         rhs=GW[:, gc, :],
                                         start=(c == 0), stop=(c == CAP - 1))
                    # agg = wsum/(ssum+1e-16) = exp(ln(wsum) - ln(ssum+eps))
                    ln_e = sb2.tile([Pn, Dn], f32, tag="lne", name="ln_e")
                    nc.scalar.activation(ln_e[:], pblk[:, 0:Dn], AF.Ln,
                                         bias=c_1e16[:])
                    ln_w = sb2.tile([Pn, Dn], f32, tag="lnw", name="ln_w")
                    nc.scalar.activation(ln_w[:], pblk[:, Dn:2 * Dn], AF.Ln,
                                         bias=c_1e30[:])
                    dlog = sb2.tile([Pn, Dn], f32, tag="dlog", name="dlog")
                    nc.vector.tensor_sub(dlog[:], ln_w[:], ln_e[:])
                    Xnm = sb2.tile([Pn, Dn], f32, tag="Xnm", name="Xnm")
                    nc.scalar.activation(Xnm[:], dlog[:], AF.Exp)
                    nc.vector.tensor_add(Xnm[:], Xnm[:], h_sb[:, b, :])
                    xT_ps = pp.tile([Pn, Dn], f32, tag="psm", name="xT_ps")
                    nc.tensor.transpose(xT_ps[:], Xnm[:], ident[:])
                    nc.scalar.activation(X_fm[:, b * Pn:(b + 1) * Pn],
                                         xT_ps[:], AF.Copy)

            # -- MLP pass 1: h1 = X@w1, BN stats over equal 250-wide tiles --
            stats6 = sb.tile([Pn, 2, NBT, 6], f32, tag="stats6",
                             name="stats6")
            for i in range(NBT):
                xs = X_fm[:, i * BTn:(i + 1) * BTn]
                for ch in range(2):
                    p1 = pp.tile([Pn, MTn], f32, tag="mm1", name="p1s")
                    nc.tensor.matmul(
                        p1[:, :BTn],
                        lhsT=w1_sb[:, l, ch * Pn:(ch + 1) * Pn],
                        rhs=xs, start=True, stop=True)
                    nc.vector.bn_stats(stats6[:, ch, i, :], p1[:, :BTn])
            mv = sb2.tile([Pn, 2, 2], f32, tag="mv", name="mv")
            for ch in range(2):
                nc.vector.bn_aggr(mv[:, ch, :], stats6[:, ch, :, :])
            # pack [mean0, mean1, ex2_0, ex2_1]
            bnar_sb = sb2.tile([Pn, 4], f32, tag="bnar", name="bnar_sb")
            nc.vector.tensor_copy(bnar_sb[:, 0:2], mv[:, :, 0])
            m2t = sb2.tile([Pn, 2], f32, tag="m2t", name="m2t")
            nc.vector.tensor_mul(m2t[:], mv[:, :, 0], mv[:, :, 0])
            nc.vector.tensor_add(bnar_sb[:, 2:4], mv[:, :, 1], m2t[:])
            bnar_in = dr.tile([Pn, 4], f32, tag="bnar_in", name="bnar_in")
            nc.sync.dma_start(out=bnar_in[:], in_=bnar_sb[:])
            bnar_out = dr.tile([Pn, 4], f32, tag="bnar_out",
                               addr_space="Shared", name="bnar_out")
            nc.gpsimd.collective_compute(
                "AllReduce", OP.add, replica_groups=rg,
                ins=[bnar_in[:]], outs=[bnar_out[:]])
            gsb = sb2.tile([Pn, 4], f32, tag="gsb", name="gsb")
            nc.sync.dma_start(out=gsb[:], in_=bnar_out[:])
            mg = sb2.tile([Pn, 2], f32, tag="mg", name="mg")
            nc.vector.tensor_scalar(out=mg[:], in0=gsb[:, 0:2],
                                    scalar1=1.0 / Wn, scalar2=None,
                                    op0=OP.mult)
            ex2 = sb2.tile([Pn, 2], f32, tag="ex2", name="ex2")
            nc.vector.tensor_scalar(out=ex2[:], in0=gsb[:, 2:4],
                                    scalar1=1.0 / Wn, scalar2=None,
                                    op0=OP.mult)
            varb = sb2.tile([Pn, 2], f32, tag="varb", name="varb")
            nc.vector.tensor_mul(varb[:], mg[:], mg[:])
            nc.vector.tensor_sub(varb[:], ex2[:], varb[:])
            lv = sb2.tile([Pn, 2], f32, tag="lv", name="lv")
            nc.scalar.activation(lv[:], varb[:], AF.Ln, bias=c_1e5[:])
            rstd = sb2.tile([Pn, 2], f32, tag="rstd", name="rstd")
            nc.scalar.activation(rstd[:], lv[:], AF.Exp, scale=-0.5)
            sc_a = sb2.tile([Pn, 2], f32, tag="sc_a", name="sc_a")
            nc.vector.tensor_mul(sc_a[:], bng_sb[:, l, :], rstd[:])
            bi_a = sb2.tile([Pn, 2], f32, tag="bi_a", name="bi_a")
            nc.vector.tensor_mul(bi_a[:], mg[:], sc_a[:])
            nc.vector.tensor_sub(bi_a[:], bnb_sb[:, l, :], bi_a[:])

            # -- MLP pass 2 + LayerNorm --
            ln_sum = sb.tile([Pn, NBn], f32, tag="ln_sum", name="ln_sum")
            ln_sq = sb.tile([Pn, NBn], f32, tag="ln_sq", name="ln_sq")
            for i in range(NMT):
                w_i = min(MTn, SHn - i * MTn)
                xs = X_fm[:, i * MTn:i * MTn + w_i]
                hbn = []
                for ch in range(2):
                    p1 = pp.tile([Pn, MTn], f32, tag="mm1", name="p1b")
                    nc.tensor.matmul(
                        p1[:, :w_i],
                        lhsT=w1_sb[:, l, ch * Pn:(ch + 1) * Pn],
                        rhs=xs, start=True, stop=True)
                    hb = sb2.tile([Pn, MTn], f32, tag=f"hbn{ch}",
                                  name="hb")
                    nc.scalar.activation(hb[:, :w_i], p1[:, :w_i], AF.Relu,
                                         scale=sc_a[:, ch:ch + 1],
                                         bias=bi_a[:, ch:ch + 1])
                    hbn.append(hb)
                p2 = pp.tile([Pn, MTn], f32, tag="mm2", name="p2")
                for ch in range(2):
                    nc.tensor.matmul(p2[:, :w_i], lhsT=w2_sb[:, l, ch, :],
                                     rhs=hbn[ch][:, :w_i],
                                     start=(ch == 0), stop=(ch == 1))
                h2c = sb2.tile([Pn, MTn], f32, tag="h2c", name="h2c")
                if p["b2_zero"]:
                    nc.scalar.activation(h2c[:, :w_i], p2[:, :w_i], AF.Copy)
                else:
                    nc.scalar.activation(h2c[:, :w_i], p2[:, :w_i],
                                         AF.Identity,
                                         bias=b2c_sb[:, l:l + 1])
                for j in range(w_i // Pn):
                    st = (i * MTn) // Pn + j
                    h2T_ps = pp.tile([Pn, Dn], f32, tag="psm", name="h2T_ps")
                    nc.tensor.transpose(h2T_ps[:],
                                        h2c[:, j * Pn:(j + 1) * Pn],
                                        ident[:])
                    nc.scalar.activation(h2T_sb[:, st, :], h2T_ps[:], AF.Copy,
                                         accum_out=ln_sum[:, st:st + 1])
                    scrap = sb2.tile([Pn, Dn], f32, tag="scrap", name="scrap")
                    nc.vector.tensor_mul(scrap[:], h2T_sb[:, st, :], h2T_ps[:])
                    scr2 = sb2.tile([Pn, Dn], f32, tag="scr2", name="scr2")
                    nc.scalar.activation(scr2[:], scrap[:], AF.Copy,
                                         accum_out=ln_sq[:, st:st + 1])
            # LN batch stats -> per-node scale A=rstd, bias B=-mu*rstd
            mu_t = sb2.tile([Pn, NBn], f32, tag="mu_t", name="mu_t")
            nc.vector.tensor_scalar(out=mu_t[:], in0=ln_sum[:],
                                    scalar1=1.0 / Dn, scalar2=None,
                                    op0=OP.mult)
            ex2t = sb2.tile([Pn, NBn], f32, tag="ex2t", name="ex2t")
            nc.vector.tensor_scalar(out=ex2t[:], in0=ln_sq[:],
                                    scalar1=1.0 / Dn, scalar2=None,
                                    op0=OP.mult)
            vart = sb2.tile([Pn, NBn], f32, tag="vart", name="vart")
            nc.vector.tensor_mul(vart[:], mu_t[:], mu_t[:])
            nc.vector.tensor_sub(vart[:], ex2t[:], vart[:])
            lvt = sb2.tile([Pn, NBn], f32, tag="lvt", name="lvt")
            nc.scalar.activation(lvt[:], vart[:], AF.Ln, bias=c_1e5[:])
            rstdt = sb2.tile([Pn, NBn], f32, tag="rstdt", name="rstdt")
            nc.scalar.activation(rstdt[:], lvt[:], AF.Exp, scale=-0.5)
            Bt = sb2.tile([Pn, NBn], f32, tag="Bt", name="Bt")
            nc.vector.tensor_scalar(out=Bt[:], in0=mu_t[:], scalar1=-1.0,
                                    scalar2=None, op0=OP.mult)
            nc.vector.tensor_mul(Bt[:], Bt[:], rstdt[:])
            for st in range(NBn):
                if p["ln_identity"]:
                    u = sb2.tile([Pn, Dn], f32, tag="u", name="u")
                    nc.scalar.activation(u[:], h2T_sb[:, st, :], AF.Relu,
                                         scale=rstdt[:, st:st + 1],
                                         bias=Bt[:, st:st + 1])
                    nc.vector.tensor_add(h_sb[:, st, :], u[:], h_sb[:, st, :])
                else:
                    u = sb2.tile([Pn, Dn], f32, tag="u", name="u")
                    nc.scalar.activation(u[:], h2T_sb[:, st, :], AF.Identity,
                                         scale=rstdt[:, st:st + 1],
                                         bias=Bt[:, st:st + 1])
                    nc.vector.tensor_mul(u[:], u[:], lng_bc[l][:])
                    nc.vector.tensor_add(u[:], u[:], lnb_bc[l][:])
                    nc.vector.tensor_scalar(out=u[:], in0=u[:], scalar1=0.0,
                                            scalar2=None, op0=OP.max)
                    nc.vector.tensor_add(h_sb[:, st, :], u[:], h_sb[:, st, :])

        # ---- out-projection: out = h @ out_w (+ out_b) ----
        out_sb = sb.tile([Pn, NBn, COUT], f32, tag="out_sb")
        for b in range(NBn):
            hT_ps = pp.tile([Pn, Dn], f32, tag="psm", name="hT_ps")
            nc.tensor.transpose(hT_ps[:], h_sb[:, b, :], ident[:])
            hT = sb2.tile([Pn, Dn], f32, tag="hT", name="hT")
            nc.scalar.activation(hT[:], hT_ps[:], AF.Copy)
            o_ps = pp.tile([Pn, COUT], f32, tag="psm", name="o_ps")
            nc.tensor.matmul(o_ps[:], lhsT=hT[:], rhs=out_w_sb[:],
                             start=True, stop=True)
            nc.scalar.activation(out_sb[:, b, :], o_ps[:], AF.Copy)
            if not p["out_b_zero"]:
                nc.vector.tensor_add(out_sb[:, b, :], out_sb[:, b, :],
                                     outb_bc[:])
        nc.sync.dma_start(
            out=out_d.rearrange("(b p) f -> p b f", p=Pn),
            in_=out_sb[:])

    _pin_act_tables()
    _fix_swdge_bump_queues(nc)
    nc.compile()
    return nc


def _fix_swdge_bump_queues(nc):
    """Tile emits the DMASW sem-bump (InstIncSwdgeSem) for prepare_only
    SWDGE preps with queue_num=0 regardless of the prep's queue. Our preps
    cycle queues exactly like Tile cycles DMASW lanes (j % 4), so lane i's
    bump belongs on queue i."""
    from concourse import bass_isa
    for b in nc.main_func.blocks:
        for i in b.instructions:
            if isinstance(i, bass_isa.InstIncSwdgeSem) and i._mode == "add":
                names = i._sem_names
                if names and names[0].startswith("DMASW"):
                    lane = int(names[0][5:].split("_")[0])
                    i.queue_num = lane % 4


def _pin_act_tables():
    """Force all activation funcs onto natural_log_exp_and_others so the
    kernel needs exactly one ACT table load (Exp/Ln/Copy/Relu/Identity are
    all members). Default placement ping-pongs exp_and_others <->
    natural_log, costing ~1.3us per switch."""
    import concourse.bacc as bacc_mod
    import concourse.hw_specs as hw_specs_mod
    if getattr(bacc_mod, "_act_tables_pinned", False):
        return
    orig = hw_specs_mod.get_activation_tables

    def pinned(arch):
        t = orig(arch)
        keep = "natural_log_exp_and_others"
        return {name: (fns if name == keep else set())
                for name, fns in t.items()}

    bacc_mod.get_activation_tables = pinned
    bacc_mod._act_tables_pinned = True


# ---------------------------------------------------------------------------
# Host-side data prep
# ---------------------------------------------------------------------------

def prep_edges(edge_index, p):
    """Group edges by (dst core, dst block), split each block's edges into a
    lo segment (src row < LOSPLIT) and a hi segment, pad each segment to a
    multiple of 128 slots (pad gather idx 0, pad one-hot col 200), and build
    the int16 wrapped gather-index tensor per group of GRP blocks."""
    Wn, Pn, NBn, GRP = p["W"], p["P"], p["NB"], p["GRP"]
    SHR, SHn, LOSPLIT = p["SH_REAL"], p["SH"], p["LOSPLIT"]
    NG = math.ceil(NBn / GRP)
    src = edge_index[0].astype(np.int64)
    dst = edge_index[1].astype(np.int64)
    src_pad = (src // SHR) * SHn + (src % SHR)
    core = dst // SHR
    dstl = dst % SHR
    blk = dstl // Pn
    col = (dstl % Pn).astype(np.float32)
    hi = (src_pad >= LOSPLIT).astype(np.int64)
    # order edges by (core, block, hi) so each segment is contiguous
    key = (core * NBn + blk) * 2 + hi
    order = np.lexsort((src_pad, key))
    counts = np.bincount(key, minlength=Wn * NBn * 2)
    cl = counts[0::2].reshape(Wn, NBn)
    ch = counts[1::2].reshape(Wn, NBn)
    CAPL = max(1, int(math.ceil(cl.max() / Pn)))
    CAPH = int(math.ceil(ch.max() / Pn))
    CAP = CAPL + CAPH
    starts = np.zeros(Wn * NBn * 2, np.int64)
    starts[1:] = np.cumsum(counts)[:-1]
    ne = len(src)
    ko = key[order]
    pos = np.arange(ne) - starts[ko]          # position within segment
    seg_cap = np.where(np.arange(Wn * NBn * 2) % 2 == 0, CAPL, CAPH) * Pn
    # slot index within the (core, block) padded layout:
    #  lo edges:   slot = pos           (< CAPL*128)
    #  hi edges:   slot = CAPL*128 + pos
    slot = pos + (ko % 2) * CAPL * Pn
    cb = ko // 2                               # core*NB + blk
    # gather index value: row within its table (lo: src_pad, hi: -LOSPLIT)
    gidx = (src_pad[order] - hi[order] * LOSPLIT).astype(np.int16)
    idxs = np.zeros((Wn * NBn, CAP * Pn), np.int16)
    colb = np.full((Wn * NBn, CAP * Pn), 200.0, np.float32)
    idxs[cb, slot] = gidx
    colb[cb, slot] = col[order]
    # one-hot column tensor: [W, NB, 128, CAP]  (chunk-transposed)
    colb = np.ascontiguousarray(
        colb.reshape(Wn, NBn, CAP, Pn).transpose(0, 1, 3, 2))
    # gather index tensor per group: [W, NG, 128, GRP*CAP*8]
    NBpad = NG * GRP
    idxs_pad = np.zeros((Wn, NBpad, CAP * Pn), np.int16)
    idxs_pad[:, :NBn] = idxs.reshape(Wn, NBn, CAP * Pn)
    idxs_pad = idxs_pad.reshape(Wn, NG, GRP, CAP * Pn)
    lo_part = idxs_pad[:, :, :, :CAPL * Pn].reshape(Wn, NG, GRP * CAPL * Pn)
    hi_part = idxs_pad[:, :, :, CAPL * Pn:].reshape(Wn, NG, GRP * CAPH * Pn)
    flat = np.concatenate([lo_part, hi_part], axis=2)  # [W, NG, GRP*CAP*128]
    nflat = flat.shape[2]
    wrapped = np.zeros((Wn, NG, 16, nflat // 16), np.int16)
    ii = np.arange(nflat)
    wrapped[:, :, ii % 16, ii // 16] = flat
    idx16 = np.ascontiguousarray(
        np.tile(wrapped, (1, 1, 8, 1)))  # [W, NG, 128, GRP*CAP*8]
    return idx16, colb, CAPL, CAPH


def prep_in_maps(inputs, p, idx16, colb):
    Wn, Pn = p["W"], p["P"]
    SHR, SHn = p["SH_REAL"], p["SH"]
    x = np.asarray(inputs["x"], np.float32)
    in_maps = []
    for k in range(Wn):
        xs = np.zeros((SHn, x.shape[1]), np.float32)
        xs[:SHR] = x[k * SHR:(k + 1) * SHR]
        m = {
            "x_fm": np.ascontiguousarray(xs.T),
            "idx16": idx16[k],
            "dst_col": colb[k],
            "in_w": np.asarray(inputs["in_w"], np.float32),
            "w1": np.asarray(inputs["w1"], np.float32),
            "w2": np.asarray(inputs["w2"], np.float32),
            "bn_g": np.asarray(inputs["bn_g"], np.float32),
            "bn_b": np.asarray(inputs["bn_b"], np.float32),
            "out_w": np.asarray(inputs["out_w"], np.float32),
        }
        if not p["b2_zero"]:
            m["b2"] = np.asarray(inputs["b2"], np.float32)
        if not p["t_one"]:
            m["t"] = np.asarray(inputs["t"], np.float32)
        if not p["in_b_zero"]:
            m["in_b"] = np.asarray(inputs["in_b"], np.float32)
        if not p["out_b_zero"]:
            m["out_b"] = np.asarray(inputs["out_b"], np.float32)
        if not p["ln_identity"]:
            m["ln_g"] = np.asarray(inputs["ln_g"], np.float32)
            m["ln_b"] = np.asarray(inputs["ln_b"], np.float32)
        in_maps.append(m)
    return in_maps


def detect_fastpath(inputs, p):
    p["t_one"] = bool(np.all(np.asarray(inputs["t"]) == 1.0))
    p["in_b_zero"] = bool(np.all(np.asarray(inputs["in_b"]) == 0.0))
    p["out_b_zero"] = bool(np.all(np.asarray(inputs["out_b"]) == 0.0))
    p["b2_zero"] = bool(np.all(np.asarray(inputs["b2"]) == 0.0))
    p["ln_identity"] = bool(
        np.all(np.asarray(inputs["ln_g"]) == 1.0)
        and np.all(np.asarray(inputs["ln_b"]) == 0.0))
    # b1 is skipped unconditionally: it cancels exactly in BatchNorm.
    return p


_PROGRAM_CACHE = {}


def _get_program(p):
    key = (p["CAPL"], p["CAPH"], p["t_one"], p["in_b_zero"],
           p["out_b_zero"], p["b2_zero"], p["ln_identity"])
    if key not in _PROGRAM_CACHE:
        _PROGRAM_CACHE[key] = build_program(p)
    return _PROGRAM_CACHE[key]


def _ensure_ntff_hook():
    """Register the axon NTFF profiling hook (the image's antenv package
    lacks axon_hooks; inject an equivalent module)."""
    import types
    if "antenv.axon_hooks" in sys.modules:
        return
    sys.path.insert(0, "/root/.axon_site")
    from trn_agent_boot.trn_boot import _ntff_profile_via_ctypes
    hook = _ntff_profile_via_ctypes("/opt/axon/libaxon_pjrt.so")
    mod = types.ModuleType("antenv.axon_hooks")
    mod._hook = hook
    mod.set_axon_ntff_profile_hook = lambda h: setattr(mod, "_hook", h)
    mod.get_axon_ntff_profile_hook = lambda: mod._hook
    sys.modules["antenv.axon_hooks"] = mod


def run(inputs, trace=False, trace_cores=None):
    from concourse.bass_utils import run_bass_kernel_spmd
    if trace:
        _ensure_ntff_hook()
    p = default_params()
    detect_fastpath(inputs, p)
    idx16, colb, CAPL, CAPH = prep_edges(
        np.asarray(inputs["edge_index"]), p)
    p["CAPL"], p["CAPH"] = CAPL, CAPH
    nc = _get_program(p)
    in_maps = prep_in_maps(inputs, p, idx16, colb)
    kwargs = {}
    if trace:
        kwargs = dict(trace=True,
                      trace_cores=trace_cores or [0])
    bkr = run_bass_kernel_spmd(nc, in_maps, core_ids=list(range(p["W"])),
                               **kwargs)
    outs = []
    for k in range(p["W"]):
        outs.append(np.asarray(bkr.results[k]["out"])[:p["SH_REAL"]])
    full = np.concatenate(outs, axis=0).astype(np.float32)
    return full, bkr


def kernel(**inputs):
    full, _ = run(inputs, trace=False)
    return full



# revision 6
# speedup vs baseline: 1.6781x; 1.6781x over previous
"""DeeperGCN (GENConv softmax-aggr, L=2) Trainium2 kernel, 8-core SPMD.

Strategy:
  - Nodes 1D-partitioned: core k owns 6250 nodes (padded to 6272 = 49*128).
  - Per layer, each core computes node-level message tables
    E = exp(t*m), Wt = m*E with m = relu(h)+eps for its shard (fp16),
    AllGathers the full [50176, 256] table, then processes its incident
    edges (grouped by dst block of 128 nodes) with:
      indirect-DMA row gather  ->  one-hot matmul scatter-accumulate in PSUM.
    softmax aggregate = wsum/ssum computed as exp(ln(wsum)-ln(ssum)).
  - MLP: w1/w2 kept stationary on PE; BatchNorm stats via bn_stats/bn_aggr
    (equal 250-wide tiles, exact combine) + AllReduce; LayerNorm per node
    after PE transpose back to node-major.
  - Dtypes: fp32 everywhere except the gathered message tables / one-hot
    matrices (fp16) whose products accumulate in fp32 PSUM.
"""

import os
import sys
import math

import numpy as np

sys.path.insert(0, "/opt/trn_rl_repo")

# Problem constants (hardcoded per contract)
N = 50000
E_EDGES = 640000
D = 128
D2 = 256
L = 2
C_IN = 128
C_OUT = 64
MSG_EPS = 1e-7
W = 8           # cores
P = 128         # partitions
SH_REAL = N // W          # 6250 real nodes per core
NB = math.ceil(SH_REAL / P)   # 49 node blocks per core
SH = NB * P               # 6272 padded nodes per core
NPAD = SH * W             # 50176
BT = 250                  # BatchNorm stats tile width (SH_REAL % BT == 0)
MT = 512                  # MLP node-tile width


def default_params():
    return dict(
        W=W, P=P, D=D, D2=D2, L=L, C_OUT=C_OUT, SH=SH, SH_REAL=SH_REAL,
        NB=NB, NPAD=NPAD, BT=BT, MT=MT, MSG_EPS=MSG_EPS,
        CAPL=10, CAPH=6, LOSPLIT=32768, GRP=2,
        # fast-path flags (host-verified against actual input values)
        t_one=True, in_b_zero=True, out_b_zero=True, ln_identity=True,
        b2_zero=True,
    )


def build_program(p):
    from concourse import bacc, bass, mybir, tile
    from concourse.bass import IndirectOffsetOnAxis
    from concourse.masks import make_identity
    from contextlib import ExitStack

    dt = mybir.dt
    f32, f16, i32 = dt.float32, dt.float16, dt.int32
    AF = mybir.ActivationFunctionType
    OP = mybir.AluOpType

    Wn, Pn, Dn, D2n = p["W"], p["P"], p["D"], p["D2"]
    Ln, COUT = p["L"], p["C_OUT"]
    SHn, SHR, NBn, NPADn = p["SH"], p["SH_REAL"], p["NB"], p["NPAD"]
    BTn, MTn = p["BT"], p["MT"]
    CAPL, CAPH, LOSPLIT, GRP = p["CAPL"], p["CAPH"], p["LOSPLIT"], p["GRP"]
    CAP = CAPL + CAPH
    NG = math.ceil(NBn / GRP)     # gather groups
    NBT = SHR // BTn              # bn stats tiles
    NMT = math.ceil(SHn / MTn)    # mlp node tiles
    eps_msg = p["MSG_EPS"]

    nc = bacc.Bacc(
        "TRN2", target_bir_lowering=False, debug=False,
        enable_asserts=False, num_devices=Wn, num_swdge_queues=4,
    )

    def din(name, shape, dty):
        return nc.dram_tensor(name, shape, dty, kind="ExternalInput").ap()

    i16 = dt.int16
    x_fm_d = din("x_fm", [Dn, SHn], f32)            # host-transposed x shard
    idx16_d = din("idx16", [NG, Pn, GRP * CAP * 8], i16)  # gather indices
    dst_col_d = din("dst_col", [NBn, Pn, CAP], f16)  # dst one-hot columns
    in_w_d = din("in_w", [Dn, Dn], f32)
    w1_d = din("w1", [Ln, Dn, D2n], f32)
    w2_d = din("w2", [Ln, D2n, Dn], f32)
    bn_g_d = din("bn_g", [Ln, D2n], f32)
    bn_b_d = din("bn_b", [Ln, D2n], f32)
    out_w_d = din("out_w", [Dn, COUT], f32)
    if not p["b2_zero"]:
        b2_d = din("b2", [Ln, Dn], f32)
    if not p["t_one"]:
        t_d = din("t", [Ln], f32)
    if not p["in_b_zero"]:
        in_b_d = din("in_b", [Dn], f32)
    if not p["out_b_zero"]:
        out_b_d = din("out_b", [COUT], f32)
    if not p["ln_identity"]:
        ln_g_d = din("ln_g", [Ln, Dn], f32)
        ln_b_d = din("ln_b", [Ln, Dn], f32)

    out_d = nc.dram_tensor("out", [SHn, COUT], f32, kind="ExternalOutput").ap()

    rg = [list(range(Wn))]

    with ExitStack() as ctx:
        tc = ctx.enter_context(tile.TileContext(nc))
        sb = ctx.enter_context(tc.tile_pool(name="sb", bufs=1))
        sb2 = ctx.enter_context(tc.tile_pool(name="sb2", bufs=2))
        pp = ctx.enter_context(tc.tile_pool(name="pp", bufs=2, space="PSUM"))
        dr = ctx.enter_context(tc.tile_pool(name="dr", bufs=2, space="DRAM"))

        # ---- constants / weights resident in SBUF ----
        ident = sb.tile([Pn, Pn], f32, tag="ident")
        make_identity(nc, ident[:])
        iota_cap = sb.tile([Pn, CAP, Pn], f16, tag="iota_cap")
        nc.gpsimd.iota(iota_cap[:], pattern=[[0, CAP], [1, Pn]], base=0,
                       channel_multiplier=0,
                       allow_small_or_imprecise_dtypes=True)

        in_w_sb = sb.tile([Pn, Dn], f32, tag="in_w")
        nc.sync.dma_start(out=in_w_sb[:], in_=in_w_d)
        w1_sb = sb.tile([Pn, Ln, D2n], f32, tag="w1")
        w2_sb = sb.tile([Pn, Ln, 2, Dn], f32, tag="w2")
        bng_sb = sb.tile([Pn, Ln, 2], f32, tag="bng")
        bnb_sb = sb.tile([Pn, Ln, 2], f32, tag="bnb")
        for l in range(Ln):
            nc.sync.dma_start(out=w1_sb[:, l, :], in_=w1_d[l])
            for ch in range(2):
                nc.sync.dma_start(out=w2_sb[:, l, ch, :],
                                  in_=w2_d[l, ch * Pn:(ch + 1) * Pn, :])
            nc.sync.dma_start(
                out=bng_sb[:, l, :],
                in_=bn_g_d[l].rearrange("(c p) -> p c", p=Pn))
            nc.sync.dma_start(
                out=bnb_sb[:, l, :],
                in_=bn_b_d[l].rearrange("(c p) -> p c", p=Pn))
        out_w_sb = sb.tile([Pn, COUT], f32, tag="out_w")
        nc.sync.dma_start(out=out_w_sb[:], in_=out_w_d)

        ones_row = sb.tile([1, Pn], f32, tag="ones_row")
        nc.vector.memset(ones_row[:], 1.0)

        def const_col(val, tagname):
            tcol = sb.tile([Pn, 1], f32, tag=tagname)
            nc.vector.memset(tcol[:], val)
            return tcol

        c_1e16 = const_col(1e-16, "c_1e16")
        c_1e30 = const_col(1e-30, "c_1e30")
        c_1e5 = const_col(1e-5, "c_1e5")

        def bcast_row(dram_row_ap, width, tagname):
            """[1,width] dram -> [128,width] sbuf via ones-matmul."""
            row = sb.tile([1, width], f32, tag=tagname + "_r")
            nc.sync.dma_start(out=row[:], in_=dram_row_ap)
            ps = pp.tile([Pn, width], f32, tag="psm", name=tagname + "_ps")
            nc.tensor.matmul(ps[:], lhsT=ones_row[:], rhs=row[:],
                             start=True, stop=True)
            out = sb.tile([Pn, width], f32, tag=tagname)
            nc.scalar.activation(out[:], ps[:], AF.Copy)
            return out

        if not p["b2_zero"]:
            b2c_sb = sb.tile([Pn, Ln], f32, tag="b2c")
            for l in range(Ln):
                nc.sync.dma_start(out=b2c_sb[:, l:l + 1], in_=b2_d[l][:, None])
        if not p["t_one"]:
            t_bc = bcast_row(t_d[None, :], Ln, "t_bc")  # [128, L]
        if not p["in_b_zero"]:
            inb_bc = bcast_row(in_b_d[None, :], Dn, "inb_bc")
        if not p["out_b_zero"]:
            outb_bc = bcast_row(out_b_d[None, :], COUT, "outb_bc")
        if not p["ln_identity"]:
            lng_bc = [bcast_row(ln_g_d[l][None, :], Dn, f"lng{l}")
                      for l in range(Ln)]
            lnb_bc = [bcast_row(ln_b_d[l][None, :], Dn, f"lnb{l}")
                      for l in range(Ln)]

        # ---- edge metadata ----
        gsems = [nc.alloc_semaphore(f"gsem{q}") for q in range(4)]
        prep_counter = [0]

        def next_q():
            q = prep_counter[0] % 4
            prep_counter[0] += 1
            return q
        dstc_sb = sb.tile([Pn, NBn, CAP], f16, tag="dstc")
        nc.sync.dma_start(out=dstc_sb[:],
                          in_=dst_col_d.rearrange("b p c -> p b c"))

        # ---- persistent state ----
        h_sb = sb.tile([Pn, NBn, Dn], f32, tag="h")      # node-major h shard
        X_fm = sb.tile([Pn, SHn], f32, tag="Xfm")        # feature-major agg+h
        h2T_sb = sb.tile([Pn, NBn, Dn], f32, tag="h2T")  # node-major h2

        # ---- in-projection: h0 = x @ in_w (+ in_b) ----
        # X_fm doubles as the staging buffer for the transposed x shard;
        # the layer-0 edge phase overwrites it only after in-proj reads it.
        nc.sync.dma_start(out=X_fm[:], in_=x_fm_d)
        for b in range(NBn):
            h0_ps = pp.tile([Pn, Dn], f32, tag="psm", name="h0_ps")
            nc.tensor.matmul(h0_ps[:], lhsT=X_fm[:, b * Pn:(b + 1) * Pn],
                             rhs=in_w_sb[:], start=True, stop=True)
            nc.scalar.activation(h_sb[:, b, :], h0_ps[:], AF.Copy)
            if not p["in_b_zero"]:
                nc.vector.tensor_add(h_sb[:, b, :], h_sb[:, b, :], inb_bc[:])

        # ---- layers ----
        for l in range(Ln):
            # -- node-level message tables: ew = [exp(t*m) | m*exp(t*m)] fp16
            ew_sb = sb.tile([Pn, NBn, 2 * Dn], f16, tag="ew", name="ew_sb")
            for b in range(NBn):
                m_sb = sb2.tile([Pn, Dn], f32, tag="m", name="m_sb")
                nc.vector.tensor_scalar(
                    out=m_sb[:], in0=h_sb[:, b, :], scalar1=0.0,
                    scalar2=eps_msg, op0=OP.max, op1=OP.add)
                e_sb = sb2.tile([Pn, Dn], f32, tag="e", name="e_sb")
                if p["t_one"]:
                    nc.scalar.activation(e_sb[:], m_sb[:], AF.Exp)
                else:
                    nc.scalar.activation(e_sb[:], m_sb[:], AF.Exp,
                                         scale=t_bc[:, l:l + 1])
                nc.vector.tensor_copy(ew_sb[:, b, 0:Dn], e_sb[:])
                nc.vector.tensor_mul(ew_sb[:, b, Dn:2 * Dn], m_sb[:], e_sb[:])

            ew_shard = dr.tile([SHn, 2 * Dn], f16, tag="ew_shard",
                               name="ew_shard")
            nc.sync.dma_start(
                out=ew_shard[:].rearrange("(b p) f -> p b f", p=Pn),
                in_=ew_sb[:])
            ew_full = dr.tile([NPADn, 2 * Dn], f16, tag="ew_full",
                              addr_space="Shared", name="ew_full")
            nc.gpsimd.collective_compute(
                "AllGather", OP.bypass, replica_groups=rg,
                ins=[ew_shard[:]], outs=[ew_full[:]])

            # -- edge aggregation: per group of GRP dst blocks, two
            #    dma_gathers (lo table rows [0,LOSPLIT), hi from LOSPLIT),
            #    then per block one-hot matmul accumulation --
            for g in range(NG):
                nblk = min(GRP, NBn - g * GRP)   # real blocks in group
                idxt = sb2.tile([Pn, GRP * CAP * 8], i16, tag="idxt",
                                name="idxt")
                nc.sync.dma_start(out=idxt[:], in_=idx16_d[g])
                GW = sb2.tile([Pn, GRP * CAP, 2 * Dn], f16, tag="gw",
                              name="GW")
                nlo = GRP * CAPL * Pn
                nc.gpsimd.dma_gather(
                    out_ap=GW[:, 0:GRP * CAPL, :], in_ap=ew_full[:],
                    idxs_ap=idxt[:, 0:GRP * CAPL * 8],
                    num_idxs=nlo, num_idxs_reg=nlo, elem_size=2 * Dn,
                    single_packet=False, queue_num=next_q())
                if CAPH > 0:
                    nhi = GRP * CAPH * Pn
                    nc.gpsimd.dma_gather(
                        out_ap=GW[:, GRP * CAPL:GRP * CAP, :],
                        in_ap=ew_full[LOSPLIT:, :],
                        idxs_ap=idxt[:, GRP * CAPL * 8:GRP * CAP * 8],
                        num_idxs=nhi, num_idxs_reg=nhi, elem_size=2 * Dn,
                        single_packet=False, queue_num=next_q())
                for s_ in range(nblk):
                    b = g * GRP + s_
                    S = sb2.tile([Pn, CAP, Dn], f16, tag="S", name="S")
                    col_b = dstc_sb[:, b, :].unsqueeze(-1).broadcast_to(
                        [Pn, CAP, Dn])
                    nc.vector.tensor_tensor(S[:], col_b, iota_cap[:],
                                            OP.is_equal)
                    pblk = pp.tile([Pn, 2 * Dn], f32, tag="pblk", name="pblk")
                    for c in range(CAP):
                        gc = (s_ * CAPL + c) if c < CAPL else (
                            GRP * CAPL + s_ * CAPH + (c - CAPL))
                        nc.tensor.matmul(pblk[:], lhsT=S[:, c, :],
                                         rhs=GW[:, gc, :],
                                         start=(c == 0), stop=(c == CAP - 1))
                    # agg = wsum/(ssum+1e-16) = exp(ln(wsum) - ln(ssum+eps))
                    ln_e = sb2.tile([Pn, Dn], f32, tag="lne", name="ln_e")
                    nc.scalar.activation(ln_e[:], pblk[:, 0:Dn], AF.Ln,
                                         bias=c_1e16[:])
                    ln_w = sb2.tile([Pn, Dn], f32, tag="lnw", name="ln_w")
                    nc.scalar.activation(ln_w[:], pblk[:, Dn:2 * Dn], AF.Ln,
                                         bias=c_1e30[:])
                    dlog = sb2.tile([Pn, Dn], f32, tag="dlog", name="dlog")
                    nc.vector.tensor_sub(dlog[:], ln_w[:], ln_e[:])
                    Xnm = sb2.tile([Pn, Dn], f32, tag="Xnm", name="Xnm")
                    nc.scalar.activation(Xnm[:], dlog[:], AF.Exp)
                    nc.vector.tensor_add(Xnm[:], Xnm[:], h_sb[:, b, :])
                    xT_ps = pp.tile([Pn, Dn], f32, tag="psm", name="xT_ps")
                    nc.tensor.transpose(xT_ps[:], Xnm[:], ident[:])
                    nc.scalar.activation(X_fm[:, b * Pn:(b + 1) * Pn],
                                         xT_ps[:], AF.Copy)

            # -- MLP pass 1: h1 = X@w1, BN stats over equal 250-wide tiles --
            stats6 = sb.tile([Pn, 2, NBT, 6], f32, tag="stats6",
                             name="stats6")
            for i in range(NBT):
                xs = X_fm[:, i * BTn:(i + 1) * BTn]
                for ch in range(2):
                    p1 = pp.tile([Pn, MTn], f32, tag="mm1", name="p1s")
                    nc.tensor.matmul(
                        p1[:, :BTn],
                        lhsT=w1_sb[:, l, ch * Pn:(ch + 1) * Pn],
                        rhs=xs, start=True, stop=True)
                    nc.vector.bn_stats(stats6[:, ch, i, :], p1[:, :BTn])
            mv = sb2.tile([Pn, 2, 2], f32, tag="mv", name="mv")
            for ch in range(2):
                nc.vector.bn_aggr(mv[:, ch, :], stats6[:, ch, :, :])
            # pack [mean0, mean1, ex2_0, ex2_1]
            bnar_sb = sb2.tile([Pn, 4], f32, tag="bnar", name="bnar_sb")
            nc.vector.tensor_copy(bnar_sb[:, 0:2], mv[:, :, 0])
            m2t = sb2.tile([Pn, 2], f32, tag="m2t", name="m2t")
            nc.vector.tensor_mul(m2t[:], mv[:, :, 0], mv[:, :, 0])
            nc.vector.tensor_add(bnar_sb[:, 2:4], mv[:, :, 1], m2t[:])
            bnar_in = dr.tile([Pn, 4], f32, tag="bnar_in", name="bnar_in")
            nc.sync.dma_start(out=bnar_in[:], in_=bnar_sb[:])
            bnar_out = dr.tile([Pn, 4], f32, tag="bnar_out",
                               addr_space="Shared", name="bnar_out")
            nc.gpsimd.collective_compute(
                "AllReduce", OP.add, replica_groups=rg,
                ins=[bnar_in[:]], outs=[bnar_out[:]])
            gsb = sb2.tile([Pn, 4], f32, tag="gsb", name="gsb")
            nc.sync.dma_start(out=gsb[:], in_=bnar_out[:])
            mg = sb2.tile([Pn, 2], f32, tag="mg", name="mg")
            nc.vector.tensor_scalar(out=mg[:], in0=gsb[:, 0:2],
                                    scalar1=1.0 / Wn, scalar2=None,
                                    op0=OP.mult)
            ex2 = sb2.tile([Pn, 2], f32, tag="ex2", name="ex2")
            nc.vector.tensor_scalar(out=ex2[:], in0=gsb[:, 2:4],
                                    scalar1=1.0 / Wn, scalar2=None,
                                    op0=OP.mult)
            varb = sb2.tile([Pn, 2], f32, tag="varb", name="varb")
            nc.vector.tensor_mul(varb[:], mg[:], mg[:])
            nc.vector.tensor_sub(varb[:], ex2[:], varb[:])
            lv = sb2.tile([Pn, 2], f32, tag="lv", name="lv")
            nc.scalar.activation(lv[:], varb[:], AF.Ln, bias=c_1e5[:])
            rstd = sb2.tile([Pn, 2], f32, tag="rstd", name="rstd")
            nc.scalar.activation(rstd[:], lv[:], AF.Exp, scale=-0.5)
            sc_a = sb2.tile([Pn, 2], f32, tag="sc_a", name="sc_a")
            nc.vector.tensor_mul(sc_a[:], bng_sb[:, l, :], rstd[:])
            bi_a = sb2.tile([Pn, 2], f32, tag="bi_a", name="bi_a")
            nc.vector.tensor_mul(bi_a[:], mg[:], sc_a[:])
            nc.vector.tensor_sub(bi_a[:], bnb_sb[:, l, :], bi_a[:])

            # -- MLP pass 2 + LayerNorm --
            ln_sum = sb.tile([Pn, NBn], f32, tag="ln_sum", name="ln_sum")
            ln_sq = sb.tile([Pn, NBn], f32, tag="ln_sq", name="ln_sq")
            for i in range(NMT):
                w_i = min(MTn, SHn - i * MTn)
                xs = X_fm[:, i * MTn:i * MTn + w_i]
                hbn = []
                for ch in range(2):
                    p1 = pp.tile([Pn, MTn], f32, tag="mm1", name="p1b")
                    nc.tensor.matmul(
                        p1[:, :w_i],
                        lhsT=w1_sb[:, l, ch * Pn:(ch + 1) * Pn],
                        rhs=xs, start=True, stop=True)
                    hb = sb2.tile([Pn, MTn], f32, tag=f"hbn{ch}",
                                  name="hb")
                    nc.scalar.activation(hb[:, :w_i], p1[:, :w_i], AF.Relu,
                                         scale=sc_a[:, ch:ch + 1],
                                         bias=bi_a[:, ch:ch + 1])
                    hbn.append(hb)
                p2 = pp.tile([Pn, MTn], f32, tag="mm2", name="p2")
                for ch in range(2):
                    nc.tensor.matmul(p2[:, :w_i], lhsT=w2_sb[:, l, ch, :],
                                     rhs=hbn[ch][:, :w_i],
                                     start=(ch == 0), stop=(ch == 1))
                h2c = sb2.tile([Pn, MTn], f32, tag="h2c", name="h2c")
                if p["b2_zero"]:
                    nc.scalar.activation(h2c[:, :w_i], p2[:, :w_i], AF.Copy)
                else:
                    nc.scalar.activation(h2c[:, :w_i], p2[:, :w_i],
                                         AF.Identity,
                                         bias=b2c_sb[:, l:l + 1])
                for j in range(w_i // Pn):
                    st = (i * MTn) // Pn + j
                    h2T_ps = pp.tile([Pn, Dn], f32, tag="psm", name="h2T_ps")
                    nc.tensor.transpose(h2T_ps[:],
                                        h2c[:, j * Pn:(j + 1) * Pn],
                                        ident[:])
                    nc.scalar.activation(h2T_sb[:, st, :], h2T_ps[:], AF.Copy,
                                         accum_out=ln_sum[:, st:st + 1])
                    scrap = sb2.tile([Pn, Dn], f32, tag="scrap", name="scrap")
                    nc.vector.tensor_mul(scrap[:], h2T_sb[:, st, :], h2T_ps[:])
                    scr2 = sb2.tile([Pn, Dn], f32, tag="scr2", name="scr2")
                    nc.scalar.activation(scr2[:], scrap[:], AF.Copy,
                                         accum_out=ln_sq[:, st:st + 1])
            # LN batch stats -> per-node scale A=rstd, bias B=-mu*rstd
            mu_t = sb2.tile([Pn, NBn], f32, tag="mu_t", name="mu_t")
            nc.vector.tensor_scalar(out=mu_t[:], in0=ln_sum[:],
                                    scalar1=1.0 / Dn, scalar2=None,
                                    op0=OP.mult)
            ex2t = sb2.tile([Pn, NBn], f32, tag="ex2t", name="ex2t")
            nc.vector.tensor_scalar(out=ex2t[:], in0=ln_sq[:],
                                    scalar1=1.0 / Dn, scalar2=None,
                                    op0=OP.mult)
            vart = sb2.tile([Pn, NBn], f32, tag="vart", name="vart")
            nc.vector.tensor_mul(vart[:], mu_t[:], mu_t[:])
            nc.vector.tensor_sub(vart[:], ex2t[:], vart[:])
            lvt = sb2.tile([Pn, NBn], f32, tag="lvt", name="lvt")
            nc.scalar.activation(lvt[:], vart[:], AF.Ln, bias=c_1e5[:])
            rstdt = sb2.tile([Pn, NBn], f32, tag="rstdt", name="rstdt")
            nc.scalar.activation(rstdt[:], lvt[:], AF.Exp, scale=-0.5)
            Bt = sb2.tile([Pn, NBn], f32, tag="Bt", name="Bt")
            nc.vector.tensor_scalar(out=Bt[:], in0=mu_t[:], scalar1=-1.0,
                                    scalar2=None, op0=OP.mult)
            nc.vector.tensor_mul(Bt[:], Bt[:], rstdt[:])
            for st in range(NBn):
                if p["ln_identity"]:
                    u = sb2.tile([Pn, Dn], f32, tag="u", name="u")
                    nc.scalar.activation(u[:], h2T_sb[:, st, :], AF.Relu,
                                         scale=rstdt[:, st:st + 1],
                                         bias=Bt[:, st:st + 1])
                    nc.vector.tensor_add(h_sb[:, st, :], u[:], h_sb[:, st, :])
                else:
                    u = sb2.tile([Pn, Dn], f32, tag="u", name="u")
                    nc.scalar.activation(u[:], h2T_sb[:, st, :], AF.Identity,
                                         scale=rstdt[:, st:st + 1],
                                         bias=Bt[:, st:st + 1])
                    nc.vector.tensor_mul(u[:], u[:], lng_bc[l][:])
                    nc.vector.tensor_add(u[:], u[:], lnb_bc[l][:])
                    nc.vector.tensor_scalar(out=u[:], in0=u[:], scalar1=0.0,
                                            scalar2=None, op0=OP.max)
                    nc.vector.tensor_add(h_sb[:, st, :], u[:], h_sb[:, st, :])

        # ---- out-projection: out = h @ out_w (+ out_b) ----
        out_sb = sb.tile([Pn, NBn, COUT], f32, tag="out_sb")
        for b in range(NBn):
            hT_ps = pp.tile([Pn, Dn], f32, tag="psm", name="hT_ps")
            nc.tensor.transpose(hT_ps[:], h_sb[:, b, :], ident[:])
            hT = sb2.tile([Pn, Dn], f32, tag="hT", name="hT")
            nc.scalar.activation(hT[:], hT_ps[:], AF.Copy)
            o_ps = pp.tile([Pn, COUT], f32, tag="psm", name="o_ps")
            nc.tensor.matmul(o_ps[:], lhsT=hT[:], rhs=out_w_sb[:],
                             start=True, stop=True)
            nc.scalar.activation(out_sb[:, b, :], o_ps[:], AF.Copy)
            if not p["out_b_zero"]:
                nc.vector.tensor_add(out_sb[:, b, :], out_sb[:, b, :],
                                     outb_bc[:])
        nc.sync.dma_start(
            out=out_d.rearrange("(b p) f -> p b f", p=Pn),
            in_=out_sb[:])

    _pin_act_tables()
    _fix_swdge_bump_queues(nc)
    nc.compile()
    return nc


def _fix_swdge_bump_queues(nc):
    """Tile emits the DMASW sem-bump (InstIncSwdgeSem) for prepare_only
    SWDGE preps with queue_num=0 regardless of the prep's queue. Our preps
    cycle queues exactly like Tile cycles DMASW lanes (j % 4), so lane i's
    bump belongs on queue i."""
    from concourse import bass_isa
    for b in nc.main_func.blocks:
        for i in b.instructions:
            if isinstance(i, bass_isa.InstIncSwdgeSem) and i._mode == "add":
                names = i._sem_names
                if names and names[0].startswith("DMASW"):
                    lane = int(names[0][5:].split("_")[0])
                    i.queue_num = lane % 4


def _pin_act_tables():
    """Force all activation funcs onto natural_log_exp_and_others so the
    kernel needs exactly one ACT table load (Exp/Ln/Copy/Relu/Identity are
    all members). Default placement ping-pongs exp_and_others <->
    natural_log, costing ~1.3us per switch."""
    import concourse.bacc as bacc_mod
    import concourse.hw_specs as hw_specs_mod
    if getattr(bacc_mod, "_act_tables_pinned", False):
        return
    orig = hw_specs_mod.get_activation_tables

    def pinned(arch):
        t = orig(arch)
        keep = "natural_log_exp_and_others"
        return {name: (fns if name == keep else set())
                for name, fns in t.items()}

    bacc_mod.get_activation_tables = pinned
    bacc_mod._act_tables_pinned = True


# ---------------------------------------------------------------------------
# Host-side data prep
# ---------------------------------------------------------------------------

def prep_edges(edge_index, p):
    """Group edges by (dst core, dst block), split each block's edges into a
    lo segment (src row < LOSPLIT) and a hi segment, pad each segment to a
    multiple of 128 slots (pad gather idx 0, pad one-hot col 200), and build
    the int16 wrapped gather-index tensor per group of GRP blocks."""
    Wn, Pn, NBn, GRP = p["W"], p["P"], p["NB"], p["GRP"]
    SHR, SHn, LOSPLIT = p["SH_REAL"], p["SH"], p["LOSPLIT"]
    NG = math.ceil(NBn / GRP)
    src = edge_index[0].astype(np.int64)
    dst = edge_index[1].astype(np.int64)
    src_pad = (src // SHR) * SHn + (src % SHR)
    core = dst // SHR
    dstl = dst % SHR
    blk = dstl // Pn
    col = (dstl % Pn).astype(np.float32)
    hi = (src_pad >= LOSPLIT).astype(np.int64)
    # order edges by (core, block, hi) so each segment is contiguous
    key = (core * NBn + blk) * 2 + hi
    order = np.lexsort((src_pad, key))
    counts = np.bincount(key, minlength=Wn * NBn * 2)
    cl = counts[0::2].reshape(Wn, NBn)
    ch = counts[1::2].reshape(Wn, NBn)
    CAPL = max(1, int(math.ceil(cl.max() / Pn)))
    CAPH = int(math.ceil(ch.max() / Pn))
    CAP = CAPL + CAPH
    starts = np.zeros(Wn * NBn * 2, np.int64)
    starts[1:] = np.cumsum(counts)[:-1]
    ne = len(src)
    ko = key[order]
    pos = np.arange(ne) - starts[ko]          # position within segment
    seg_cap = np.where(np.arange(Wn * NBn * 2) % 2 == 0, CAPL, CAPH) * Pn
    # slot index within the (core, block) padded layout:
    #  lo edges:   slot = pos           (< CAPL*128)
    #  hi edges:   slot = CAPL*128 + pos
    slot = pos + (ko % 2) * CAPL * Pn
    cb = ko // 2                               # core*NB + blk
    # gather index value: row within its table (lo: src_pad, hi: -LOSPLIT)
    gidx = (src_pad[order] - hi[order] * LOSPLIT).astype(np.int16)
    idxs = np.zeros((Wn * NBn, CAP * Pn), np.int16)
    colb = np.full((Wn * NBn, CAP * Pn), 200.0, np.float32)
    idxs[cb, slot] = gidx
    colb[cb, slot] = col[order]
    # one-hot column tensor: [W, NB, 128, CAP]  (chunk-transposed)
    colb = np.ascontiguousarray(
        colb.reshape(Wn, NBn, CAP, Pn).transpose(0, 1, 3, 2)).astype(
            np.float16)
    # gather index tensor per group: [W, NG, 128, GRP*CAP*8]
    NBpad = NG * GRP
    idxs_pad = np.zeros((Wn, NBpad, CAP * Pn), np.int16)
    idxs_pad[:, :NBn] = idxs.reshape(Wn, NBn, CAP * Pn)
    idxs_pad = idxs_pad.reshape(Wn, NG, GRP, CAP * Pn)
    lo_part = idxs_pad[:, :, :, :CAPL * Pn].reshape(Wn, NG, GRP * CAPL * Pn)
    hi_part = idxs_pad[:, :, :, CAPL * Pn:].reshape(Wn, NG, GRP * CAPH * Pn)
    flat = np.concatenate([lo_part, hi_part], axis=2)  # [W, NG, GRP*CAP*128]
    nflat = flat.shape[2]
    wrapped = np.zeros((Wn, NG, 16, nflat // 16), np.int16)
    ii = np.arange(nflat)
    wrapped[:, :, ii % 16, ii // 16] = flat
    idx16 = np.ascontiguousarray(
        np.tile(wrapped, (1, 1, 8, 1)))  # [W, NG, 128, GRP*CAP*8]
    return idx16, colb, CAPL, CAPH


def prep_in_maps(inputs, p, idx16, colb):
    Wn, Pn = p["W"], p["P"]
    SHR, SHn = p["SH_REAL"], p["SH"]
    x = np.asarray(inputs["x"], np.float32)
    in_maps = []
    for k in range(Wn):
        xs = np.zeros((SHn, x.shape[1]), np.float32)
        xs[:SHR] = x[k * SHR:(k + 1) * SHR]
        m = {
            "x_fm": np.ascontiguousarray(xs.T),
            "idx16": idx16[k],
            "dst_col": colb[k],
            "in_w": np.asarray(inputs["in_w"], np.float32),
            "w1": np.asarray(inputs["w1"], np.float32),
            "w2": np.asarray(inputs["w2"], np.float32),
            "bn_g": np.asarray(inputs["bn_g"], np.float32),
            "bn_b": np.asarray(inputs["bn_b"], np.float32),
            "out_w": np.asarray(inputs["out_w"], np.float32),
        }
        if not p["b2_zero"]:
            m["b2"] = np.asarray(inputs["b2"], np.float32)
        if not p["t_one"]:
            m["t"] = np.asarray(inputs["t"], np.float32)
        if not p["in_b_zero"]:
            m["in_b"] = np.asarray(inputs["in_b"], np.float32)
        if not p["out_b_zero"]:
            m["out_b"] = np.asarray(inputs["out_b"], np.float32)
        if not p["ln_identity"]:
            m["ln_g"] = np.asarray(inputs["ln_g"], np.float32)
            m["ln_b"] = np.asarray(inputs["ln_b"], np.float32)
        in_maps.append(m)
    return in_maps


def detect_fastpath(inputs, p):
    p["t_one"] = bool(np.all(np.asarray(inputs["t"]) == 1.0))
    p["in_b_zero"] = bool(np.all(np.asarray(inputs["in_b"]) == 0.0))
    p["out_b_zero"] = bool(np.all(np.asarray(inputs["out_b"]) == 0.0))
    p["b2_zero"] = bool(np.all(np.asarray(inputs["b2"]) == 0.0))
    p["ln_identity"] = bool(
        np.all(np.asarray(inputs["ln_g"]) == 1.0)
        and np.all(np.asarray(inputs["ln_b"]) == 0.0))
    # b1 is skipped unconditionally: it cancels exactly in BatchNorm.
    return p


_PROGRAM_CACHE = {}


def _get_program(p):
    key = (p["CAPL"], p["CAPH"], p["t_one"], p["in_b_zero"],
           p["out_b_zero"], p["b2_zero"], p["ln_identity"])
    if key not in _PROGRAM_CACHE:
        _PROGRAM_CACHE[key] = build_program(p)
    return _PROGRAM_CACHE[key]


def _ensure_ntff_hook():
    """Register the axon NTFF profiling hook (the image's antenv package
    lacks axon_hooks; inject an equivalent module)."""
    import types
    if "antenv.axon_hooks" in sys.modules:
        return
    sys.path.insert(0, "/root/.axon_site")
    from trn_agent_boot.trn_boot import _ntff_profile_via_ctypes
    hook = _ntff_profile_via_ctypes("/opt/axon/libaxon_pjrt.so")
    mod = types.ModuleType("antenv.axon_hooks")
    mod._hook = hook
    mod.set_axon_ntff_profile_hook = lambda h: setattr(mod, "_hook", h)
    mod.get_axon_ntff_profile_hook = lambda: mod._hook
    sys.modules["antenv.axon_hooks"] = mod


def run(inputs, trace=False, trace_cores=None):
    from concourse.bass_utils import run_bass_kernel_spmd
    if trace:
        _ensure_ntff_hook()
    p = default_params()
    detect_fastpath(inputs, p)
    idx16, colb, CAPL, CAPH = prep_edges(
        np.asarray(inputs["edge_index"]), p)
    p["CAPL"], p["CAPH"] = CAPL, CAPH
    nc = _get_program(p)
    in_maps = prep_in_maps(inputs, p, idx16, colb)
    kwargs = {}
    if trace:
        kwargs = dict(trace=True,
                      trace_cores=trace_cores or [0])
    bkr = run_bass_kernel_spmd(nc, in_maps, core_ids=list(range(p["W"])),
                               **kwargs)
    outs = []
    for k in range(p["W"]):
        outs.append(np.asarray(bkr.results[k]["out"])[:p["SH_REAL"]])
    full = np.concatenate(outs, axis=0).astype(np.float32)
    return full, bkr


def kernel(**inputs):
    full, _ = run(inputs, trace=False)
    return full



# revision 8
# speedup vs baseline: 1.7763x; 1.0585x over previous
"""DeeperGCN (GENConv softmax-aggr, L=2) Trainium2 kernel, 8-core SPMD.

Strategy:
  - Nodes 1D-partitioned: core k owns 6250 nodes (padded to 6272 = 49*128).
  - Per layer, each core computes node-level message tables
    E = exp(t*m), Wt = m*E with m = relu(h)+eps for its shard (fp16),
    AllGathers the full [50176, 256] table, then processes its incident
    edges (grouped by dst block of 128 nodes) with:
      indirect-DMA row gather  ->  one-hot matmul scatter-accumulate in PSUM.
    softmax aggregate = wsum/ssum computed as exp(ln(wsum)-ln(ssum)).
  - MLP: w1/w2 kept stationary on PE; BatchNorm stats via bn_stats/bn_aggr
    (equal 250-wide tiles, exact combine) + AllReduce; LayerNorm per node
    after PE transpose back to node-major.
  - Dtypes: fp32 everywhere except the gathered message tables / one-hot
    matrices (fp16) whose products accumulate in fp32 PSUM.
"""

import os
import sys
import math

import numpy as np

sys.path.insert(0, "/opt/trn_rl_repo")

# Problem constants (hardcoded per contract)
N = 50000
E_EDGES = 640000
D = 128
D2 = 256
L = 2
C_IN = 128
C_OUT = 64
MSG_EPS = 1e-7
W = 8           # cores
P = 128         # partitions
SH_REAL = N // W          # 6250 real nodes per core
NB = math.ceil(SH_REAL / P)   # 49 node blocks per core
SH = NB * P               # 6272 padded nodes per core
NPAD = SH * W             # 50176
BT = 250                  # BatchNorm stats tile width (SH_REAL % BT == 0)
MT = 512                  # MLP node-tile width


def default_params():
    return dict(
        W=W, P=P, D=D, D2=D2, L=L, C_OUT=C_OUT, SH=SH, SH_REAL=SH_REAL,
        NB=NB, NPAD=NPAD, BT=BT, MT=MT, MSG_EPS=MSG_EPS,
        CAPL=10, CAPH=6, LOSPLIT=32768, GRP=2,
        # fast-path flags (host-verified against actual input values)
        t_one=True, in_b_zero=True, out_b_zero=True, ln_identity=True,
        b2_zero=True,
    )


def build_program(p):
    from concourse import bacc, bass, mybir, tile
    from concourse.bass import IndirectOffsetOnAxis
    from concourse.masks import make_identity
    from contextlib import ExitStack

    dt = mybir.dt
    f32, f16, i32 = dt.float32, dt.float16, dt.int32
    AF = mybir.ActivationFunctionType
    OP = mybir.AluOpType

    Wn, Pn, Dn, D2n = p["W"], p["P"], p["D"], p["D2"]
    Ln, COUT = p["L"], p["C_OUT"]
    SHn, SHR, NBn, NPADn = p["SH"], p["SH_REAL"], p["NB"], p["NPAD"]
    BTn, MTn = p["BT"], p["MT"]
    CAPL, CAPH, LOSPLIT, GRP = p["CAPL"], p["CAPH"], p["LOSPLIT"], p["GRP"]
    CAP = CAPL + CAPH
    NG = math.ceil(NBn / GRP)     # gather groups
    NBT = SHR // BTn              # bn stats tiles
    NMT = math.ceil(SHn / MTn)    # mlp node tiles
    eps_msg = p["MSG_EPS"]

    nc = bacc.Bacc(
        "TRN2", target_bir_lowering=False, debug=False,
        enable_asserts=False, num_devices=Wn, num_swdge_queues=4,
    )

    def din(name, shape, dty):
        return nc.dram_tensor(name, shape, dty, kind="ExternalInput").ap()

    i16 = dt.int16
    x_fm_d = din("x_fm", [Dn, SHn], f32)            # host-transposed x shard
    idx16_d = din("idx16", [NG, Pn, GRP * CAP * 8], i16)  # gather indices
    dst_col_d = din("dst_col", [NBn, Pn, CAP], f16)  # dst one-hot columns
    in_w_d = din("in_w", [Dn, Dn], f32)
    w1_d = din("w1", [Ln, Dn, D2n], f32)
    w2_d = din("w2", [Ln, D2n, Dn], f32)
    bn_g_d = din("bn_g", [Ln, D2n], f32)
    bn_b_d = din("bn_b", [Ln, D2n], f32)
    out_w_d = din("out_w", [Dn, COUT], f32)
    if not p["b2_zero"]:
        b2_d = din("b2", [Ln, Dn], f32)
    if not p["t_one"]:
        t_d = din("t", [Ln], f32)
    if not p["in_b_zero"]:
        in_b_d = din("in_b", [Dn], f32)
    if not p["out_b_zero"]:
        out_b_d = din("out_b", [COUT], f32)
    if not p["ln_identity"]:
        ln_g_d = din("ln_g", [Ln, Dn], f32)
        ln_b_d = din("ln_b", [Ln, Dn], f32)

    out_d = nc.dram_tensor("out", [SHn, COUT], f32, kind="ExternalOutput").ap()

    rg = [list(range(Wn))]

    with ExitStack() as ctx:
        tc = ctx.enter_context(tile.TileContext(nc))
        sb = ctx.enter_context(tc.tile_pool(name="sb", bufs=1))
        sb2 = ctx.enter_context(tc.tile_pool(name="sb2", bufs=2))
        pp = ctx.enter_context(tc.tile_pool(name="pp", bufs=2, space="PSUM"))
        dr = ctx.enter_context(tc.tile_pool(name="dr", bufs=2, space="DRAM"))

        # ---- constants / weights resident in SBUF ----
        ident = sb.tile([Pn, Pn], f32, tag="ident")
        make_identity(nc, ident[:])
        iota_cap = sb.tile([Pn, CAP, Pn], f16, tag="iota_cap")
        nc.gpsimd.iota(iota_cap[:], pattern=[[0, CAP], [1, Pn]], base=0,
                       channel_multiplier=0,
                       allow_small_or_imprecise_dtypes=True)

        in_w_sb = sb.tile([Pn, Dn], f32, tag="in_w")
        nc.sync.dma_start(out=in_w_sb[:], in_=in_w_d)
        w1_sb = sb.tile([Pn, Ln, D2n], f32, tag="w1")
        w2_sb = sb.tile([Pn, Ln, 2, Dn], f32, tag="w2")
        bng_sb = sb.tile([Pn, Ln, 2], f32, tag="bng")
        bnb_sb = sb.tile([Pn, Ln, 2], f32, tag="bnb")
        for l in range(Ln):
            nc.sync.dma_start(out=w1_sb[:, l, :], in_=w1_d[l])
            for ch in range(2):
                nc.sync.dma_start(out=w2_sb[:, l, ch, :],
                                  in_=w2_d[l, ch * Pn:(ch + 1) * Pn, :])
            nc.sync.dma_start(
                out=bng_sb[:, l, :],
                in_=bn_g_d[l].rearrange("(c p) -> p c", p=Pn))
            nc.sync.dma_start(
                out=bnb_sb[:, l, :],
                in_=bn_b_d[l].rearrange("(c p) -> p c", p=Pn))
        out_w_sb = sb.tile([Pn, COUT], f32, tag="out_w")
        nc.sync.dma_start(out=out_w_sb[:], in_=out_w_d)

        ones_row = sb.tile([1, Pn], f32, tag="ones_row")
        nc.vector.memset(ones_row[:], 1.0)

        def const_col(val, tagname):
            tcol = sb.tile([Pn, 1], f32, tag=tagname)
            nc.vector.memset(tcol[:], val)
            return tcol

        c_1e16 = const_col(1e-16, "c_1e16")
        c_1e30 = const_col(1e-30, "c_1e30")
        c_1e5 = const_col(1e-5, "c_1e5")

        def bcast_row(dram_row_ap, width, tagname):
            """[1,width] dram -> [128,width] sbuf via ones-matmul."""
            row = sb.tile([1, width], f32, tag=tagname + "_r")
            nc.sync.dma_start(out=row[:], in_=dram_row_ap)
            ps = pp.tile([Pn, width], f32, tag="psm", name=tagname + "_ps")
            nc.tensor.matmul(ps[:], lhsT=ones_row[:], rhs=row[:],
                             start=True, stop=True)
            out = sb.tile([Pn, width], f32, tag=tagname)
            nc.scalar.activation(out[:], ps[:], AF.Copy)
            return out

        if not p["b2_zero"]:
            b2c_sb = sb.tile([Pn, Ln], f32, tag="b2c")
            for l in range(Ln):
                nc.sync.dma_start(out=b2c_sb[:, l:l + 1], in_=b2_d[l][:, None])
        if not p["t_one"]:
            t_bc = bcast_row(t_d[None, :], Ln, "t_bc")  # [128, L]
        if not p["in_b_zero"]:
            inb_bc = bcast_row(in_b_d[None, :], Dn, "inb_bc")
        if not p["out_b_zero"]:
            outb_bc = bcast_row(out_b_d[None, :], COUT, "outb_bc")
        if not p["ln_identity"]:
            lng_bc = [bcast_row(ln_g_d[l][None, :], Dn, f"lng{l}")
                      for l in range(Ln)]
            lnb_bc = [bcast_row(ln_b_d[l][None, :], Dn, f"lnb{l}")
                      for l in range(Ln)]

        # ---- edge metadata ----
        gsems = [nc.alloc_semaphore(f"gsem{q}") for q in range(4)]
        prep_counter = [0]

        def next_q():
            q = prep_counter[0] % 4
            prep_counter[0] += 1
            return q
        dstc_sb = sb.tile([Pn, NBn, CAP], f16, tag="dstc")
        nc.sync.dma_start(out=dstc_sb[:],
                          in_=dst_col_d.rearrange("b p c -> p b c"))

        # ---- persistent state ----
        h_sb = sb.tile([Pn, NBn, Dn], f32, tag="h")      # node-major h shard
        X_fm = sb.tile([Pn, SHn], f32, tag="Xfm")        # feature-major agg+h
        h2T_sb = sb.tile([Pn, NBn, Dn], f32, tag="h2T")  # node-major h2

        # ---- in-projection: h0 = x @ in_w (+ in_b) ----
        # X_fm doubles as the staging buffer for the transposed x shard;
        # the layer-0 edge phase overwrites it only after in-proj reads it.
        nc.sync.dma_start(out=X_fm[:], in_=x_fm_d)
        for b in range(NBn):
            h0_ps = pp.tile([Pn, Dn], f32, tag="psm", name="h0_ps")
            nc.tensor.matmul(h0_ps[:], lhsT=X_fm[:, b * Pn:(b + 1) * Pn],
                             rhs=in_w_sb[:], start=True, stop=True)
            nc.scalar.activation(h_sb[:, b, :], h0_ps[:], AF.Copy)
            if not p["in_b_zero"]:
                nc.vector.tensor_add(h_sb[:, b, :], h_sb[:, b, :], inb_bc[:])

        # ---- layers ----
        for l in range(Ln):
            # -- node-level message table: m = relu(h) + eps  (fp16)
            m_tab = sb.tile([Pn, NBn, Dn], f16, tag="mtab", name="m_tab")
            for b in range(NBn):
                nc.vector.tensor_scalar(
                    out=m_tab[:, b, :], in0=h_sb[:, b, :], scalar1=0.0,
                    scalar2=eps_msg, op0=OP.max, op1=OP.add)

            ew_shard = dr.tile([SHn, Dn], f16, tag="ew_shard",
                               name="ew_shard")
            nc.sync.dma_start(
                out=ew_shard[:].rearrange("(b p) f -> p b f", p=Pn),
                in_=m_tab[:])
            ew_full = dr.tile([NPADn, Dn], f16, tag="ew_full",
                              addr_space="Shared", name="ew_full")
            nc.gpsimd.collective_compute(
                "AllGather", OP.bypass, replica_groups=rg,
                ins=[ew_shard[:]], outs=[ew_full[:]])

            # -- edge aggregation: per group of GRP dst blocks, two
            #    dma_gathers (lo table rows [0,LOSPLIT), hi from LOSPLIT),
            #    then per block one-hot matmul accumulation --
            for g in range(NG):
                nblk = min(GRP, NBn - g * GRP)   # real blocks in group
                idxt = sb2.tile([Pn, GRP * CAP * 8], i16, tag="idxt",
                                name="idxt")
                nc.sync.dma_start(out=idxt[:], in_=idx16_d[g])
                GM = sb2.tile([Pn, GRP * CAP, Dn], f16, tag="gm",
                              name="GM")
                nlo = GRP * CAPL * Pn
                nc.gpsimd.dma_gather(
                    out_ap=GM[:, 0:GRP * CAPL, :], in_ap=ew_full[:],
                    idxs_ap=idxt[:, 0:GRP * CAPL * 8],
                    num_idxs=nlo, num_idxs_reg=nlo, elem_size=Dn,
                    single_packet=False, queue_num=next_q())
                if CAPH > 0:
                    nhi = GRP * CAPH * Pn
                    nc.gpsimd.dma_gather(
                        out_ap=GM[:, GRP * CAPL:GRP * CAP, :],
                        in_ap=ew_full[LOSPLIT:, :],
                        idxs_ap=idxt[:, GRP * CAPL * 8:GRP * CAP * 8],
                        num_idxs=nhi, num_idxs_reg=nhi, elem_size=Dn,
                        single_packet=False, queue_num=next_q())
                # expand to [E | m*E] per slot:  E = exp(t*m), W = m*E
                GW = sb2.tile([Pn, GRP * CAP, 2 * Dn], f16, tag="gw",
                              name="GW")
                if p["t_one"]:
                    nc.scalar.activation(GW[:, :, 0:Dn], GM[:], AF.Exp)
                else:
                    nc.scalar.activation(GW[:, :, 0:Dn], GM[:], AF.Exp,
                                         scale=t_bc[:, l:l + 1])
                nc.vector.tensor_mul(GW[:, :, Dn:2 * Dn], GM[:],
                                     GW[:, :, 0:Dn])
                for s_ in range(nblk):
                    b = g * GRP + s_
                    S = sb2.tile([Pn, CAP, Dn], f16, tag="S", name="S")
                    col_b = dstc_sb[:, b, :].unsqueeze(-1).broadcast_to(
                        [Pn, CAP, Dn])
                    nc.vector.tensor_tensor(S[:], col_b, iota_cap[:],
                                            OP.is_equal)
                    pblk = pp.tile([Pn, 2 * Dn], f32, tag="pblk", name="pblk")
                    for c in range(CAP):
                        gc = (s_ * CAPL + c) if c < CAPL else (
                            GRP * CAPL + s_ * CAPH + (c - CAPL))
                        nc.tensor.matmul(pblk[:], lhsT=S[:, c, :],
                                         rhs=GW[:, gc, :],
                                         start=(c == 0), stop=(c == CAP - 1))
                    # agg = wsum/(ssum+1e-16) = exp(ln(wsum) - ln(ssum+eps))
                    ln_e = sb2.tile([Pn, Dn], f32, tag="lne", name="ln_e")
                    nc.scalar.activation(ln_e[:], pblk[:, 0:Dn], AF.Ln,
                                         bias=c_1e16[:])
                    ln_w = sb2.tile([Pn, Dn], f32, tag="lnw", name="ln_w")
                    nc.scalar.activation(ln_w[:], pblk[:, Dn:2 * Dn], AF.Ln,
                                         bias=c_1e30[:])
                    dlog = sb2.tile([Pn, Dn], f32, tag="dlog", name="dlog")
                    nc.vector.tensor_sub(dlog[:], ln_w[:], ln_e[:])
                    Xnm = sb2.tile([Pn, Dn], f32, tag="Xnm", name="Xnm")
                    nc.scalar.activation(Xnm[:], dlog[:], AF.Exp)
                    nc.vector.tensor_add(Xnm[:], Xnm[:], h_sb[:, b, :])
                    xT_ps = pp.tile([Pn, Dn], f32, tag="psm", name="xT_ps")
                    nc.tensor.transpose(xT_ps[:], Xnm[:], ident[:])
                    nc.scalar.activation(X_fm[:, b * Pn:(b + 1) * Pn],
                                         xT_ps[:], AF.Copy)

            # -- MLP pass 1: h1 = X@w1, BN stats over equal 250-wide tiles --
            stats6 = sb.tile([Pn, 2, NBT, 6], f32, tag="stats6",
                             name="stats6")
            for i in range(NBT):
                xs = X_fm[:, i * BTn:(i + 1) * BTn]
                for ch in range(2):
                    p1 = pp.tile([Pn, MTn], f32, tag="mm1", name="p1s")
                    nc.tensor.matmul(
                        p1[:, :BTn],
                        lhsT=w1_sb[:, l, ch * Pn:(ch + 1) * Pn],
                        rhs=xs, start=True, stop=True)
                    nc.vector.bn_stats(stats6[:, ch, i, :], p1[:, :BTn])
            mv = sb2.tile([Pn, 2, 2], f32, tag="mv", name="mv")
            for ch in range(2):
                nc.vector.bn_aggr(mv[:, ch, :], stats6[:, ch, :, :])
            # pack [mean0, mean1, ex2_0, ex2_1]
            bnar_sb = sb2.tile([Pn, 4], f32, tag="bnar", name="bnar_sb")
            nc.vector.tensor_copy(bnar_sb[:, 0:2], mv[:, :, 0])
            m2t = sb2.tile([Pn, 2], f32, tag="m2t", name="m2t")
            nc.vector.tensor_mul(m2t[:], mv[:, :, 0], mv[:, :, 0])
            nc.vector.tensor_add(bnar_sb[:, 2:4], mv[:, :, 1], m2t[:])
            bnar_in = dr.tile([Pn, 4], f32, tag="bnar_in", name="bnar_in")
            nc.sync.dma_start(out=bnar_in[:], in_=bnar_sb[:])
            bnar_out = dr.tile([Pn, 4], f32, tag="bnar_out",
                               addr_space="Shared", name="bnar_out")
            nc.gpsimd.collective_compute(
                "AllReduce", OP.add, replica_groups=rg,
                ins=[bnar_in[:]], outs=[bnar_out[:]])
            gsb = sb2.tile([Pn, 4], f32, tag="gsb", name="gsb")
            nc.sync.dma_start(out=gsb[:], in_=bnar_out[:])
            mg = sb2.tile([Pn, 2], f32, tag="mg", name="mg")
            nc.vector.tensor_scalar(out=mg[:], in0=gsb[:, 0:2],
                                    scalar1=1.0 / Wn, scalar2=None,
                                    op0=OP.mult)
            ex2 = sb2.tile([Pn, 2], f32, tag="ex2", name="ex2")
            nc.vector.tensor_scalar(out=ex2[:], in0=gsb[:, 2:4],
                                    scalar1=1.0 / Wn, scalar2=None,
                                    op0=OP.mult)
            varb = sb2.tile([Pn, 2], f32, tag="varb", name="varb")
            nc.vector.tensor_mul(varb[:], mg[:], mg[:])
            nc.vector.tensor_sub(varb[:], ex2[:], varb[:])
            lv = sb2.tile([Pn, 2], f32, tag="lv", name="lv")
            nc.scalar.activation(lv[:], varb[:], AF.Ln, bias=c_1e5[:])
            rstd = sb2.tile([Pn, 2], f32, tag="rstd", name="rstd")
            nc.scalar.activation(rstd[:], lv[:], AF.Exp, scale=-0.5)
            sc_a = sb2.tile([Pn, 2], f32, tag="sc_a", name="sc_a")
            nc.vector.tensor_mul(sc_a[:], bng_sb[:, l, :], rstd[:])
            bi_a = sb2.tile([Pn, 2], f32, tag="bi_a", name="bi_a")
            nc.vector.tensor_mul(bi_a[:], mg[:], sc_a[:])
            nc.vector.tensor_sub(bi_a[:], bnb_sb[:, l, :], bi_a[:])

            # -- MLP pass 2 + LayerNorm --
            ln_sum = sb.tile([Pn, NBn], f32, tag="ln_sum", name="ln_sum")
            ln_sq = sb.tile([Pn, NBn], f32, tag="ln_sq", name="ln_sq")
            for i in range(NMT):
                w_i = min(MTn, SHn - i * MTn)
                xs = X_fm[:, i * MTn:i * MTn + w_i]
                hbn = []
                for ch in range(2):
                    p1 = pp.tile([Pn, MTn], f32, tag="mm1", name="p1b")
                    nc.tensor.matmul(
                        p1[:, :w_i],
                        lhsT=w1_sb[:, l, ch * Pn:(ch + 1) * Pn],
                        rhs=xs, start=True, stop=True)
                    hb = sb2.tile([Pn, MTn], f32, tag=f"hbn{ch}",
                                  name="hb")
                    nc.scalar.activation(hb[:, :w_i], p1[:, :w_i], AF.Relu,
                                         scale=sc_a[:, ch:ch + 1],
                                         bias=bi_a[:, ch:ch + 1])
                    hbn.append(hb)
                p2 = pp.tile([Pn, MTn], f32, tag="mm2", name="p2")
                for ch in range(2):
                    nc.tensor.matmul(p2[:, :w_i], lhsT=w2_sb[:, l, ch, :],
                                     rhs=hbn[ch][:, :w_i],
                                     start=(ch == 0), stop=(ch == 1))
                h2c = sb2.tile([Pn, MTn], f32, tag="h2c", name="h2c")
                if p["b2_zero"]:
                    nc.scalar.activation(h2c[:, :w_i], p2[:, :w_i], AF.Copy)
                else:
                    nc.scalar.activation(h2c[:, :w_i], p2[:, :w_i],
                                         AF.Identity,
                                         bias=b2c_sb[:, l:l + 1])
                for j in range(w_i // Pn):
                    st = (i * MTn) // Pn + j
                    h2T_ps = pp.tile([Pn, Dn], f32, tag="psm", name="h2T_ps")
                    nc.tensor.transpose(h2T_ps[:],
                                        h2c[:, j * Pn:(j + 1) * Pn],
                                        ident[:])
                    nc.scalar.activation(h2T_sb[:, st, :], h2T_ps[:], AF.Copy,
                                         accum_out=ln_sum[:, st:st + 1])
                    scrap = sb2.tile([Pn, Dn], f32, tag="scrap", name="scrap")
                    nc.vector.tensor_mul(scrap[:], h2T_sb[:, st, :], h2T_ps[:])
                    scr2 = sb2.tile([Pn, Dn], f32, tag="scr2", name="scr2")
                    nc.scalar.activation(scr2[:], scrap[:], AF.Copy,
                                         accum_out=ln_sq[:, st:st + 1])
            # LN batch stats -> per-node scale A=rstd, bias B=-mu*rstd
            mu_t = sb2.tile([Pn, NBn], f32, tag="mu_t", name="mu_t")
            nc.vector.tensor_scalar(out=mu_t[:], in0=ln_sum[:],
                                    scalar1=1.0 / Dn, scalar2=None,
                                    op0=OP.mult)
            ex2t = sb2.tile([Pn, NBn], f32, tag="ex2t", name="ex2t")
            nc.vector.tensor_scalar(out=ex2t[:], in0=ln_sq[:],
                                    scalar1=1.0 / Dn, scalar2=None,
                                    op0=OP.mult)
            vart = sb2.tile([Pn, NBn], f32, tag="vart", name="vart")
            nc.vector.tensor_mul(vart[:], mu_t[:], mu_t[:])
            nc.vector.tensor_sub(vart[:], ex2t[:], vart[:])
            lvt = sb2.tile([Pn, NBn], f32, tag="lvt", name="lvt")
            nc.scalar.activation(lvt[:], vart[:], AF.Ln, bias=c_1e5[:])
            rstdt = sb2.tile([Pn, NBn], f32, tag="rstdt", name="rstdt")
            nc.scalar.activation(rstdt[:], lvt[:], AF.Exp, scale=-0.5)
            Bt = sb2.tile([Pn, NBn], f32, tag="Bt", name="Bt")
            nc.vector.tensor_scalar(out=Bt[:], in0=mu_t[:], scalar1=-1.0,
                                    scalar2=None, op0=OP.mult)
            nc.vector.tensor_mul(Bt[:], Bt[:], rstdt[:])
            for st in range(NBn):
                if p["ln_identity"]:
                    u = sb2.tile([Pn, Dn], f32, tag="u", name="u")
                    nc.scalar.activation(u[:], h2T_sb[:, st, :], AF.Relu,
                                         scale=rstdt[:, st:st + 1],
                                         bias=Bt[:, st:st + 1])
                    nc.vector.tensor_add(h_sb[:, st, :], u[:], h_sb[:, st, :])
                else:
                    u = sb2.tile([Pn, Dn], f32, tag="u", name="u")
                    nc.scalar.activation(u[:], h2T_sb[:, st, :], AF.Identity,
                                         scale=rstdt[:, st:st + 1],
                                         bias=Bt[:, st:st + 1])
                    nc.vector.tensor_mul(u[:], u[:], lng_bc[l][:])
                    nc.vector.tensor_add(u[:], u[:], lnb_bc[l][:])
                    nc.vector.tensor_scalar(out=u[:], in0=u[:], scalar1=0.0,
                                            scalar2=None, op0=OP.max)
                    nc.vector.tensor_add(h_sb[:, st, :], u[:], h_sb[:, st, :])

        # ---- out-projection: out = h @ out_w (+ out_b) ----
        out_sb = sb.tile([Pn, NBn, COUT], f32, tag="out_sb")
        for b in range(NBn):
            hT_ps = pp.tile([Pn, Dn], f32, tag="psm", name="hT_ps")
            nc.tensor.transpose(hT_ps[:], h_sb[:, b, :], ident[:])
            hT = sb2.tile([Pn, Dn], f32, tag="hT", name="hT")
            nc.scalar.activation(hT[:], hT_ps[:], AF.Copy)
            o_ps = pp.tile([Pn, COUT], f32, tag="psm", name="o_ps")
            nc.tensor.matmul(o_ps[:], lhsT=hT[:], rhs=out_w_sb[:],
                             start=True, stop=True)
            nc.scalar.activation(out_sb[:, b, :], o_ps[:], AF.Copy)
            if not p["out_b_zero"]:
                nc.vector.tensor_add(out_sb[:, b, :], out_sb[:, b, :],
                                     outb_bc[:])
        nc.sync.dma_start(
            out=out_d.rearrange("(b p) f -> p b f", p=Pn),
            in_=out_sb[:])

    _pin_act_tables()
    _fix_swdge_bump_queues(nc)
    nc.compile()
    return nc


def _fix_swdge_bump_queues(nc):
    """Tile emits the DMASW sem-bump (InstIncSwdgeSem) for prepare_only
    SWDGE preps with queue_num=0 regardless of the prep's queue. Our preps
    cycle queues exactly like Tile cycles DMASW lanes (j % 4), so lane i's
    bump belongs on queue i."""
    from concourse import bass_isa
    for b in nc.main_func.blocks:
        for i in b.instructions:
            if isinstance(i, bass_isa.InstIncSwdgeSem) and i._mode == "add":
                names = i._sem_names
                if names and names[0].startswith("DMASW"):
                    lane = int(names[0][5:].split("_")[0])
                    i.queue_num = lane % 4


def _pin_act_tables():
    """Force all activation funcs onto natural_log_exp_and_others so the
    kernel needs exactly one ACT table load (Exp/Ln/Copy/Relu/Identity are
    all members). Default placement ping-pongs exp_and_others <->
    natural_log, costing ~1.3us per switch."""
    import concourse.bacc as bacc_mod
    import concourse.hw_specs as hw_specs_mod
    if getattr(bacc_mod, "_act_tables_pinned", False):
        return
    orig = hw_specs_mod.get_activation_tables

    def pinned(arch):
        t = orig(arch)
        keep = "natural_log_exp_and_others"
        return {name: (fns if name == keep else set())
                for name, fns in t.items()}

    bacc_mod.get_activation_tables = pinned
    bacc_mod._act_tables_pinned = True


# ---------------------------------------------------------------------------
# Host-side data prep
# ---------------------------------------------------------------------------

def prep_edges(edge_index, p):
    """Group edges by (dst core, dst block), split each block's edges into a
    lo segment (src row < LOSPLIT) and a hi segment, pad each segment to a
    multiple of 128 slots (pad gather idx 0, pad one-hot col 200), and build
    the int16 wrapped gather-index tensor per group of GRP blocks."""
    Wn, Pn, NBn, GRP = p["W"], p["P"], p["NB"], p["GRP"]
    SHR, SHn, LOSPLIT = p["SH_REAL"], p["SH"], p["LOSPLIT"]
    NG = math.ceil(NBn / GRP)
    src = edge_index[0].astype(np.int64)
    dst = edge_index[1].astype(np.int64)
    src_pad = (src // SHR) * SHn + (src % SHR)
    core = dst // SHR
    dstl = dst % SHR
    blk = dstl // Pn
    col = (dstl % Pn).astype(np.float32)
    hi = (src_pad >= LOSPLIT).astype(np.int64)
    # order edges by (core, block, hi) so each segment is contiguous
    key = (core * NBn + blk) * 2 + hi
    order = np.lexsort((src_pad, key))
    counts = np.bincount(key, minlength=Wn * NBn * 2)
    cl = counts[0::2].reshape(Wn, NBn)
    ch = counts[1::2].reshape(Wn, NBn)
    CAPL = max(1, int(math.ceil(cl.max() / Pn)))
    CAPH = int(math.ceil(ch.max() / Pn))
    CAP = CAPL + CAPH
    starts = np.zeros(Wn * NBn * 2, np.int64)
    starts[1:] = np.cumsum(counts)[:-1]
    ne = len(src)
    ko = key[order]
    pos = np.arange(ne) - starts[ko]          # position within segment
    seg_cap = np.where(np.arange(Wn * NBn * 2) % 2 == 0, CAPL, CAPH) * Pn
    # slot index within the (core, block) padded layout:
    #  lo edges:   slot = pos           (< CAPL*128)
    #  hi edges:   slot = CAPL*128 + pos
    slot = pos + (ko % 2) * CAPL * Pn
    cb = ko // 2                               # core*NB + blk
    # gather index value: row within its table (lo: src_pad, hi: -LOSPLIT)
    gidx = (src_pad[order] - hi[order] * LOSPLIT).astype(np.int16)
    idxs = np.zeros((Wn * NBn, CAP * Pn), np.int16)
    colb = np.full((Wn * NBn, CAP * Pn), 200.0, np.float32)
    idxs[cb, slot] = gidx
    colb[cb, slot] = col[order]
    # one-hot column tensor: [W, NB, 128, CAP]  (chunk-transposed)
    colb = np.ascontiguousarray(
        colb.reshape(Wn, NBn, CAP, Pn).transpose(0, 1, 3, 2)).astype(
            np.float16)
    # gather index tensor per group: [W, NG, 128, GRP*CAP*8]
    NBpad = NG * GRP
    idxs_pad = np.zeros((Wn, NBpad, CAP * Pn), np.int16)
    idxs_pad[:, :NBn] = idxs.reshape(Wn, NBn, CAP * Pn)
    idxs_pad = idxs_pad.reshape(Wn, NG, GRP, CAP * Pn)
    lo_part = idxs_pad[:, :, :, :CAPL * Pn].reshape(Wn, NG, GRP * CAPL * Pn)
    hi_part = idxs_pad[:, :, :, CAPL * Pn:].reshape(Wn, NG, GRP * CAPH * Pn)
    flat = np.concatenate([lo_part, hi_part], axis=2)  # [W, NG, GRP*CAP*128]
    nflat = flat.shape[2]
    wrapped = np.zeros((Wn, NG, 16, nflat // 16), np.int16)
    ii = np.arange(nflat)
    wrapped[:, :, ii % 16, ii // 16] = flat
    idx16 = np.ascontiguousarray(
        np.tile(wrapped, (1, 1, 8, 1)))  # [W, NG, 128, GRP*CAP*8]
    return idx16, colb, CAPL, CAPH


def prep_in_maps(inputs, p, idx16, colb):
    Wn, Pn = p["W"], p["P"]
    SHR, SHn = p["SH_REAL"], p["SH"]
    x = np.asarray(inputs["x"], np.float32)
    in_maps = []
    for k in range(Wn):
        xs = np.zeros((SHn, x.shape[1]), np.float32)
        xs[:SHR] = x[k * SHR:(k + 1) * SHR]
        m = {
            "x_fm": np.ascontiguousarray(xs.T),
            "idx16": idx16[k],
            "dst_col": colb[k],
            "in_w": np.asarray(inputs["in_w"], np.float32),
            "w1": np.asarray(inputs["w1"], np.float32),
            "w2": np.asarray(inputs["w2"], np.float32),
            "bn_g": np.asarray(inputs["bn_g"], np.float32),
            "bn_b": np.asarray(inputs["bn_b"], np.float32),
            "out_w": np.asarray(inputs["out_w"], np.float32),
        }
        if not p["b2_zero"]:
            m["b2"] = np.asarray(inputs["b2"], np.float32)
        if not p["t_one"]:
            m["t"] = np.asarray(inputs["t"], np.float32)
        if not p["in_b_zero"]:
            m["in_b"] = np.asarray(inputs["in_b"], np.float32)
        if not p["out_b_zero"]:
            m["out_b"] = np.asarray(inputs["out_b"], np.float32)
        if not p["ln_identity"]:
            m["ln_g"] = np.asarray(inputs["ln_g"], np.float32)
            m["ln_b"] = np.asarray(inputs["ln_b"], np.float32)
        in_maps.append(m)
    return in_maps


def detect_fastpath(inputs, p):
    p["t_one"] = bool(np.all(np.asarray(inputs["t"]) == 1.0))
    p["in_b_zero"] = bool(np.all(np.asarray(inputs["in_b"]) == 0.0))
    p["out_b_zero"] = bool(np.all(np.asarray(inputs["out_b"]) == 0.0))
    p["b2_zero"] = bool(np.all(np.asarray(inputs["b2"]) == 0.0))
    p["ln_identity"] = bool(
        np.all(np.asarray(inputs["ln_g"]) == 1.0)
        and np.all(np.asarray(inputs["ln_b"]) == 0.0))
    # b1 is skipped unconditionally: it cancels exactly in BatchNorm.
    return p


_PROGRAM_CACHE = {}


def _get_program(p):
    key = (p["CAPL"], p["CAPH"], p["t_one"], p["in_b_zero"],
           p["out_b_zero"], p["b2_zero"], p["ln_identity"])
    if key not in _PROGRAM_CACHE:
        _PROGRAM_CACHE[key] = build_program(p)
    return _PROGRAM_CACHE[key]


def _ensure_ntff_hook():
    """Register the axon NTFF profiling hook (the image's antenv package
    lacks axon_hooks; inject an equivalent module)."""
    import types
    if "antenv.axon_hooks" in sys.modules:
        return
    sys.path.insert(0, "/root/.axon_site")
    from trn_agent_boot.trn_boot import _ntff_profile_via_ctypes
    hook = _ntff_profile_via_ctypes("/opt/axon/libaxon_pjrt.so")
    mod = types.ModuleType("antenv.axon_hooks")
    mod._hook = hook
    mod.set_axon_ntff_profile_hook = lambda h: setattr(mod, "_hook", h)
    mod.get_axon_ntff_profile_hook = lambda: mod._hook
    sys.modules["antenv.axon_hooks"] = mod


def run(inputs, trace=False, trace_cores=None):
    from concourse.bass_utils import run_bass_kernel_spmd
    if trace:
        _ensure_ntff_hook()
    p = default_params()
    detect_fastpath(inputs, p)
    idx16, colb, CAPL, CAPH = prep_edges(
        np.asarray(inputs["edge_index"]), p)
    p["CAPL"], p["CAPH"] = CAPL, CAPH
    nc = _get_program(p)
    in_maps = prep_in_maps(inputs, p, idx16, colb)
    kwargs = {}
    if trace:
        kwargs = dict(trace=True,
                      trace_cores=trace_cores or [0])
    bkr = run_bass_kernel_spmd(nc, in_maps, core_ids=list(range(p["W"])),
                               **kwargs)
    outs = []
    for k in range(p["W"]):
        outs.append(np.asarray(bkr.results[k]["out"])[:p["SH_REAL"]])
    full = np.concatenate(outs, axis=0).astype(np.float32)
    return full, bkr


def kernel(**inputs):
    full, _ = run(inputs, trace=False)
    return full



# revision 11
# speedup vs baseline: 1.9048x; 1.0723x over previous
"""DeeperGCN (GENConv softmax-aggr, L=2) Trainium2 kernel, 8-core SPMD.

Strategy:
  - Nodes 1D-partitioned: core k owns 6250 nodes (padded to 6272 = 49*128).
  - Per layer, each core computes node-level message tables
    E = exp(t*m), Wt = m*E with m = relu(h)+eps for its shard (fp16),
    AllGathers the full [50176, 256] table, then processes its incident
    edges (grouped by dst block of 128 nodes) with:
      indirect-DMA row gather  ->  one-hot matmul scatter-accumulate in PSUM.
    softmax aggregate = wsum/ssum computed as exp(ln(wsum)-ln(ssum)).
  - MLP: w1/w2 kept stationary on PE; BatchNorm stats via bn_stats/bn_aggr
    (equal 250-wide tiles, exact combine) + AllReduce; LayerNorm per node
    after PE transpose back to node-major.
  - Dtypes: fp32 everywhere except the gathered message tables / one-hot
    matrices (fp16) whose products accumulate in fp32 PSUM.
"""

import os
import sys
import math

import numpy as np

sys.path.insert(0, "/opt/trn_rl_repo")

# Problem constants (hardcoded per contract)
N = 50000
E_EDGES = 640000
D = 128
D2 = 256
L = 2
C_IN = 128
C_OUT = 64
MSG_EPS = 1e-7
W = 8           # cores
P = 128         # partitions
SH_REAL = N // W          # 6250 real nodes per core
NB = math.ceil(SH_REAL / P)   # 49 node blocks per core
SH = NB * P               # 6272 padded nodes per core
NPAD = SH * W             # 50176
BT = 250                  # BatchNorm stats tile width (SH_REAL % BT == 0)
MT = 512                  # MLP node-tile width


def default_params():
    return dict(
        W=W, P=P, D=D, D2=D2, L=L, C_OUT=C_OUT, SH=SH, SH_REAL=SH_REAL,
        NB=NB, NPAD=NPAD, BT=BT, MT=MT, MSG_EPS=MSG_EPS,
        CAPL=10, CAPH=6, LOSPLIT=32768, GRP=2,
        # fast-path flags (host-verified against actual input values)
        t_one=True, in_b_zero=True, out_b_zero=True, ln_identity=True,
        b2_zero=True,
    )


def build_program(p):
    from concourse import bacc, bass, mybir, tile
    from concourse.bass import IndirectOffsetOnAxis
    from concourse.masks import make_identity
    from contextlib import ExitStack

    dt = mybir.dt
    f32, f16, i32 = dt.float32, dt.float16, dt.int32
    AF = mybir.ActivationFunctionType
    OP = mybir.AluOpType

    Wn, Pn, Dn, D2n = p["W"], p["P"], p["D"], p["D2"]
    Ln, COUT = p["L"], p["C_OUT"]
    SHn, SHR, NBn, NPADn = p["SH"], p["SH_REAL"], p["NB"], p["NPAD"]
    BTn, MTn = p["BT"], p["MT"]
    CAPL, CAPH, LOSPLIT, GRP = p["CAPL"], p["CAPH"], p["LOSPLIT"], p["GRP"]
    CAP = CAPL + CAPH
    NG = math.ceil(NBn / GRP)     # gather groups
    NBT = SHR // BTn              # bn stats tiles
    NMT = math.ceil(SHn / MTn)    # mlp node tiles
    eps_msg = p["MSG_EPS"]

    nc = bacc.Bacc(
        "TRN2", target_bir_lowering=False, debug=False,
        enable_asserts=False, num_devices=Wn, num_swdge_queues=4,
    )

    def din(name, shape, dty):
        return nc.dram_tensor(name, shape, dty, kind="ExternalInput").ap()

    i16 = dt.int16
    x_fm_d = din("x_fm", [Dn, SHn], f32)            # host-transposed x shard
    idx16_d = din("idx16", [NG, Pn, GRP * CAP * 8], i16)  # gather indices
    dst_col_d = din("dst_col", [NBn, Pn, CAP], f16)  # dst one-hot columns
    in_w_d = din("in_w", [Dn, Dn], f32)
    w1_d = din("w1", [Ln, Dn, D2n], f32)
    w2_d = din("w2", [Ln, D2n, Dn], f32)
    bn_g_d = din("bn_g", [Ln, D2n], f32)
    bn_b_d = din("bn_b", [Ln, D2n], f32)
    out_w_d = din("out_w", [Dn, COUT], f32)
    if not p["b2_zero"]:
        b2_d = din("b2", [Ln, Dn], f32)
    if not p["t_one"]:
        t_d = din("t", [Ln], f32)
    if not p["in_b_zero"]:
        in_b_d = din("in_b", [Dn], f32)
    if not p["out_b_zero"]:
        out_b_d = din("out_b", [COUT], f32)
    if not p["ln_identity"]:
        ln_g_d = din("ln_g", [Ln, Dn], f32)
        ln_b_d = din("ln_b", [Ln, Dn], f32)

    out_d = nc.dram_tensor("out", [SHn, COUT], f32, kind="ExternalOutput").ap()

    rg = [list(range(Wn))]

    with ExitStack() as ctx:
        tc = ctx.enter_context(tile.TileContext(nc))
        sb = ctx.enter_context(tc.tile_pool(name="sb", bufs=1))
        sb2 = ctx.enter_context(tc.tile_pool(name="sb2", bufs=2))
        pp = ctx.enter_context(tc.tile_pool(name="pp", bufs=2, space="PSUM"))
        dr = ctx.enter_context(tc.tile_pool(name="dr", bufs=2, space="DRAM"))

        # ---- constants / weights resident in SBUF ----
        ident = sb.tile([Pn, Pn], f32, tag="ident")
        make_identity(nc, ident[:])
        iota_cap = sb.tile([Pn, CAP, Pn], f16, tag="iota_cap")
        nc.gpsimd.iota(iota_cap[:], pattern=[[0, CAP], [1, Pn]], base=0,
                       channel_multiplier=0,
                       allow_small_or_imprecise_dtypes=True)

        in_w_sb = sb.tile([Pn, Dn], f32, tag="in_w")
        nc.sync.dma_start(out=in_w_sb[:], in_=in_w_d)
        w1_sb = sb.tile([Pn, Ln, D2n], f32, tag="w1")
        w2_sb = sb.tile([Pn, Ln, 2, Dn], f32, tag="w2")
        bng_sb = sb.tile([Pn, Ln, 2], f32, tag="bng")
        bnb_sb = sb.tile([Pn, Ln, 2], f32, tag="bnb")
        for l in range(Ln):
            nc.sync.dma_start(out=w1_sb[:, l, :], in_=w1_d[l])
            for ch in range(2):
                nc.sync.dma_start(out=w2_sb[:, l, ch, :],
                                  in_=w2_d[l, ch * Pn:(ch + 1) * Pn, :])
            nc.sync.dma_start(
                out=bng_sb[:, l, :],
                in_=bn_g_d[l].rearrange("(c p) -> p c", p=Pn))
            nc.sync.dma_start(
                out=bnb_sb[:, l, :],
                in_=bn_b_d[l].rearrange("(c p) -> p c", p=Pn))
        out_w_sb = sb.tile([Pn, COUT], f32, tag="out_w")
        nc.sync.dma_start(out=out_w_sb[:], in_=out_w_d)

        ones_row = sb.tile([1, Pn], f32, tag="ones_row")
        nc.vector.memset(ones_row[:], 1.0)

        def const_col(val, tagname):
            tcol = sb.tile([Pn, 1], f32, tag=tagname)
            nc.vector.memset(tcol[:], val)
            return tcol

        c_1e16 = const_col(1e-16, "c_1e16")
        c_1e30 = const_col(1e-30, "c_1e30")
        c_1e5 = const_col(1e-5, "c_1e5")

        def bcast_row(dram_row_ap, width, tagname):
            """[1,width] dram -> [128,width] sbuf via ones-matmul."""
            row = sb.tile([1, width], f32, tag=tagname + "_r")
            nc.sync.dma_start(out=row[:], in_=dram_row_ap)
            ps = pp.tile([Pn, width], f32, tag="psm", name=tagname + "_ps")
            nc.tensor.matmul(ps[:], lhsT=ones_row[:], rhs=row[:],
                             start=True, stop=True)
            out = sb.tile([Pn, width], f32, tag=tagname)
            nc.scalar.activation(out[:], ps[:], AF.Copy)
            return out

        if not p["b2_zero"]:
            b2c_sb = sb.tile([Pn, Ln], f32, tag="b2c")
            for l in range(Ln):
                nc.sync.dma_start(out=b2c_sb[:, l:l + 1], in_=b2_d[l][:, None])
        if not p["t_one"]:
            t_bc = bcast_row(t_d[None, :], Ln, "t_bc")  # [128, L]
        if not p["in_b_zero"]:
            inb_bc = bcast_row(in_b_d[None, :], Dn, "inb_bc")
        if not p["out_b_zero"]:
            outb_bc = bcast_row(out_b_d[None, :], COUT, "outb_bc")
        if not p["ln_identity"]:
            lng_bc = [bcast_row(ln_g_d[l][None, :], Dn, f"lng{l}")
                      for l in range(Ln)]
            lnb_bc = [bcast_row(ln_b_d[l][None, :], Dn, f"lnb{l}")
                      for l in range(Ln)]

        # ---- edge metadata ----
        gsems = [nc.alloc_semaphore(f"gsem{q}") for q in range(4)]
        prep_counter = [0]

        def next_q():
            q = prep_counter[0] % 4
            prep_counter[0] += 1
            return q
        dstc_sb = sb.tile([Pn, NBn, CAP], f16, tag="dstc")
        nc.sync.dma_start(out=dstc_sb[:],
                          in_=dst_col_d.rearrange("b p c -> p b c"))

        # ---- persistent state ----
        h_sb = sb.tile([Pn, NBn, Dn], f32, tag="h")      # node-major h shard
        X_fm = sb.tile([Pn, SHn], f32, tag="Xfm")        # feature-major agg+h
        h2T_sb = sb.tile([Pn, NBn, Dn], f32, tag="h2T")  # node-major h2

        # ---- in-projection: h0 = x @ in_w (+ in_b) ----
        # X_fm doubles as the staging buffer for the transposed x shard;
        # the layer-0 edge phase overwrites it only after in-proj reads it.
        nc.sync.dma_start(out=X_fm[:], in_=x_fm_d)
        for b in range(NBn):
            h0_ps = pp.tile([Pn, Dn], f32, tag="psm", name="h0_ps")
            nc.tensor.matmul(h0_ps[:], lhsT=X_fm[:, b * Pn:(b + 1) * Pn],
                             rhs=in_w_sb[:], start=True, stop=True)
            nc.scalar.activation(h_sb[:, b, :], h0_ps[:], AF.Copy)
            if not p["in_b_zero"]:
                nc.vector.tensor_add(h_sb[:, b, :], h_sb[:, b, :], inb_bc[:])

        # ---- layers ----
        for l in range(Ln):
            # -- node-level message table: m = relu(h) + eps  (fp16)
            m_tab = sb.tile([Pn, NBn, Dn], f16, tag="mtab", name="m_tab")
            for b in range(NBn):
                nc.vector.tensor_scalar(
                    out=m_tab[:, b, :], in0=h_sb[:, b, :], scalar1=0.0,
                    scalar2=eps_msg, op0=OP.max, op1=OP.add)

            ew_shard = dr.tile([SHn, Dn], f16, tag="ew_shard",
                               name="ew_shard")
            nc.sync.dma_start(
                out=ew_shard[:].rearrange("(b p) f -> p b f", p=Pn),
                in_=m_tab[:])
            ew_full = dr.tile([NPADn, Dn], f16, tag="ew_full",
                              addr_space="Shared", name="ew_full")
            nc.gpsimd.collective_compute(
                "AllGather", OP.bypass, replica_groups=rg,
                ins=[ew_shard[:]], outs=[ew_full[:]])

            # -- edge aggregation: per group of GRP dst blocks, two
            #    dma_gathers (lo table rows [0,LOSPLIT), hi from LOSPLIT),
            #    then per block one-hot matmul accumulation --
            for g in range(NG):
                nblk = min(GRP, NBn - g * GRP)   # real blocks in group
                idxt = sb2.tile([Pn, GRP * CAP * 8], i16, tag="idxt",
                                name="idxt")
                nc.sync.dma_start(out=idxt[:], in_=idx16_d[g])
                GM = sb2.tile([Pn, GRP * CAP, Dn], f16, tag="gm",
                              name="GM")

                def emit_gathers(col0, ncols, table_ap, woff):
                    # split into <=16-column (2048-idx) chunks: larger
                    # gathers hit a slow ucode emission path (~8.6 vs
                    # ~3.4 ns/idx measured)
                    parts = max(1, math.ceil(ncols / 12))
                    cc0 = math.ceil(ncols / parts)
                    done = 0
                    while done < ncols:
                        cc = min(cc0, ncols - done)
                        nn = cc * Pn
                        nc.gpsimd.dma_gather(
                            out_ap=GM[:, col0 + done:col0 + done + cc, :],
                            in_ap=table_ap,
                            idxs_ap=idxt[:, woff + done * 8:
                                         woff + (done + cc) * 8],
                            num_idxs=nn, num_idxs_reg=nn, elem_size=Dn,
                            single_packet=False, queue_num=next_q())
                        done += cc

                emit_gathers(0, GRP * CAPL, ew_full[:], 0)
                if CAPH > 0:
                    emit_gathers(GRP * CAPL, GRP * CAPH, ew_full[LOSPLIT:, :],
                                 GRP * CAPL * 8)
                # expand to [E | m*E] per slot:  E = exp(t*m), W = m*E
                GW = sb2.tile([Pn, GRP * CAP, 2 * Dn], f16, tag="gw",
                              name="GW")
                if p["t_one"]:
                    nc.scalar.activation(GW[:, :, 0:Dn], GM[:], AF.Exp)
                else:
                    nc.scalar.activation(GW[:, :, 0:Dn], GM[:], AF.Exp,
                                         scale=t_bc[:, l:l + 1])
                nc.vector.tensor_mul(GW[:, :, Dn:2 * Dn], GM[:],
                                     GW[:, :, 0:Dn])
                for s_ in range(nblk):
                    b = g * GRP + s_
                    S = sb2.tile([Pn, CAP, Dn], f16, tag="S", name="S")
                    col_b = dstc_sb[:, b, :].unsqueeze(-1).broadcast_to(
                        [Pn, CAP, Dn])
                    nc.vector.tensor_tensor(S[:], col_b, iota_cap[:],
                                            OP.is_equal)
                    pblk = pp.tile([Pn, 2 * Dn], f32, tag="pblk", name="pblk")
                    for c in range(CAP):
                        gc = (s_ * CAPL + c) if c < CAPL else (
                            GRP * CAPL + s_ * CAPH + (c - CAPL))
                        nc.tensor.matmul(pblk[:], lhsT=S[:, c, :],
                                         rhs=GW[:, gc, :],
                                         start=(c == 0), stop=(c == CAP - 1))
                    # agg = wsum/(ssum+1e-16) = exp(ln(wsum) - ln(ssum+eps))
                    ln_e = sb2.tile([Pn, Dn], f32, tag="lne", name="ln_e")
                    nc.scalar.activation(ln_e[:], pblk[:, 0:Dn], AF.Ln,
                                         bias=c_1e16[:])
                    ln_w = sb2.tile([Pn, Dn], f32, tag="lnw", name="ln_w")
                    nc.scalar.activation(ln_w[:], pblk[:, Dn:2 * Dn], AF.Ln,
                                         bias=c_1e30[:])
                    dlog = sb2.tile([Pn, Dn], f32, tag="dlog", name="dlog")
                    nc.vector.tensor_sub(dlog[:], ln_w[:], ln_e[:])
                    Xnm = sb2.tile([Pn, Dn], f32, tag="Xnm", name="Xnm")
                    nc.scalar.activation(Xnm[:], dlog[:], AF.Exp)
                    nc.vector.tensor_add(Xnm[:], Xnm[:], h_sb[:, b, :])
                    xT_ps = pp.tile([Pn, Dn], f32, tag="psm", name="xT_ps")
                    nc.tensor.transpose(xT_ps[:], Xnm[:], ident[:])
                    nc.scalar.activation(X_fm[:, b * Pn:(b + 1) * Pn],
                                         xT_ps[:], AF.Copy)

            # -- MLP pass 1: h1 = X@w1, BN stats over equal 250-wide tiles --
            stats6 = sb.tile([Pn, 2, NBT, 6], f32, tag="stats6",
                             name="stats6")
            for i in range(NBT):
                xs = X_fm[:, i * BTn:(i + 1) * BTn]
                for ch in range(2):
                    p1 = pp.tile([Pn, MTn], f32, tag="mm1", name="p1s")
                    nc.tensor.matmul(
                        p1[:, :BTn],
                        lhsT=w1_sb[:, l, ch * Pn:(ch + 1) * Pn],
                        rhs=xs, start=True, stop=True)
                    nc.vector.bn_stats(stats6[:, ch, i, :], p1[:, :BTn])
            mv = sb2.tile([Pn, 2, 2], f32, tag="mv", name="mv")
            for ch in range(2):
                nc.vector.bn_aggr(mv[:, ch, :], stats6[:, ch, :, :])
            # pack [mean0, mean1, ex2_0, ex2_1]
            bnar_sb = sb2.tile([Pn, 4], f32, tag="bnar", name="bnar_sb")
            nc.vector.tensor_copy(bnar_sb[:, 0:2], mv[:, :, 0])
            m2t = sb2.tile([Pn, 2], f32, tag="m2t", name="m2t")
            nc.vector.tensor_mul(m2t[:], mv[:, :, 0], mv[:, :, 0])
            nc.vector.tensor_add(bnar_sb[:, 2:4], mv[:, :, 1], m2t[:])
            bnar_in = dr.tile([Pn, 4], f32, tag="bnar_in", name="bnar_in")
            nc.sync.dma_start(out=bnar_in[:], in_=bnar_sb[:])
            bnar_out = dr.tile([Pn, 4], f32, tag="bnar_out",
                               addr_space="Shared", name="bnar_out")
            nc.gpsimd.collective_compute(
                "AllReduce", OP.add, replica_groups=rg,
                ins=[bnar_in[:]], outs=[bnar_out[:]])
            gsb = sb2.tile([Pn, 4], f32, tag="gsb", name="gsb")
            nc.sync.dma_start(out=gsb[:], in_=bnar_out[:])
            mg = sb2.tile([Pn, 2], f32, tag="mg", name="mg")
            nc.vector.tensor_scalar(out=mg[:], in0=gsb[:, 0:2],
                                    scalar1=1.0 / Wn, scalar2=None,
                                    op0=OP.mult)
            ex2 = sb2.tile([Pn, 2], f32, tag="ex2", name="ex2")
            nc.vector.tensor_scalar(out=ex2[:], in0=gsb[:, 2:4],
                                    scalar1=1.0 / Wn, scalar2=None,
                                    op0=OP.mult)
            varb = sb2.tile([Pn, 2], f32, tag="varb", name="varb")
            nc.vector.tensor_mul(varb[:], mg[:], mg[:])
            nc.vector.tensor_sub(varb[:], ex2[:], varb[:])
            lv = sb2.tile([Pn, 2], f32, tag="lv", name="lv")
            nc.scalar.activation(lv[:], varb[:], AF.Ln, bias=c_1e5[:])
            rstd = sb2.tile([Pn, 2], f32, tag="rstd", name="rstd")
            nc.scalar.activation(rstd[:], lv[:], AF.Exp, scale=-0.5)
            sc_a = sb2.tile([Pn, 2], f32, tag="sc_a", name="sc_a")
            nc.vector.tensor_mul(sc_a[:], bng_sb[:, l, :], rstd[:])
            bi_a = sb2.tile([Pn, 2], f32, tag="bi_a", name="bi_a")
            nc.vector.tensor_mul(bi_a[:], mg[:], sc_a[:])
            nc.vector.tensor_sub(bi_a[:], bnb_sb[:, l, :], bi_a[:])

            # -- MLP pass 2 + LayerNorm --
            ln_sum = sb.tile([Pn, NBn], f32, tag="ln_sum", name="ln_sum")
            ln_sq = sb.tile([Pn, NBn], f32, tag="ln_sq", name="ln_sq")
            for i in range(NMT):
                w_i = min(MTn, SHn - i * MTn)
                xs = X_fm[:, i * MTn:i * MTn + w_i]
                hbn = []
                for ch in range(2):
                    p1 = pp.tile([Pn, MTn], f32, tag="mm1", name="p1b")
                    nc.tensor.matmul(
                        p1[:, :w_i],
                        lhsT=w1_sb[:, l, ch * Pn:(ch + 1) * Pn],
                        rhs=xs, start=True, stop=True)
                    hb = sb2.tile([Pn, MTn], f32, tag=f"hbn{ch}",
                                  name="hb")
                    nc.scalar.activation(hb[:, :w_i], p1[:, :w_i], AF.Relu,
                                         scale=sc_a[:, ch:ch + 1],
                                         bias=bi_a[:, ch:ch + 1])
                    hbn.append(hb)
                p2 = pp.tile([Pn, MTn], f32, tag="mm2", name="p2")
                for ch in range(2):
                    nc.tensor.matmul(p2[:, :w_i], lhsT=w2_sb[:, l, ch, :],
                                     rhs=hbn[ch][:, :w_i],
                                     start=(ch == 0), stop=(ch == 1))
                h2c = sb2.tile([Pn, MTn], f32, tag="h2c", name="h2c")
                if p["b2_zero"]:
                    nc.scalar.activation(h2c[:, :w_i], p2[:, :w_i], AF.Copy)
                else:
                    nc.scalar.activation(h2c[:, :w_i], p2[:, :w_i],
                                         AF.Identity,
                                         bias=b2c_sb[:, l:l + 1])
                for j in range(w_i // Pn):
                    st = (i * MTn) // Pn + j
                    h2T_ps = pp.tile([Pn, Dn], f32, tag="psm", name="h2T_ps")
                    nc.tensor.transpose(h2T_ps[:],
                                        h2c[:, j * Pn:(j + 1) * Pn],
                                        ident[:])
                    nc.scalar.activation(h2T_sb[:, st, :], h2T_ps[:], AF.Copy,
                                         accum_out=ln_sum[:, st:st + 1])
                    scrap = sb2.tile([Pn, Dn], f32, tag="scrap", name="scrap")
                    nc.vector.tensor_mul(scrap[:], h2T_sb[:, st, :], h2T_ps[:])
                    scr2 = sb2.tile([Pn, Dn], f32, tag="scr2", name="scr2")
                    nc.scalar.activation(scr2[:], scrap[:], AF.Copy,
                                         accum_out=ln_sq[:, st:st + 1])
            # LN batch stats -> per-node scale A=rstd, bias B=-mu*rstd
            mu_t = sb2.tile([Pn, NBn], f32, tag="mu_t", name="mu_t")
            nc.vector.tensor_scalar(out=mu_t[:], in0=ln_sum[:],
                                    scalar1=1.0 / Dn, scalar2=None,
                                    op0=OP.mult)
            ex2t = sb2.tile([Pn, NBn], f32, tag="ex2t", name="ex2t")
            nc.vector.tensor_scalar(out=ex2t[:], in0=ln_sq[:],
                                    scalar1=1.0 / Dn, scalar2=None,
                                    op0=OP.mult)
            vart = sb2.tile([Pn, NBn], f32, tag="vart", name="vart")
            nc.vector.tensor_mul(vart[:], mu_t[:], mu_t[:])
            nc.vector.tensor_sub(vart[:], ex2t[:], vart[:])
            lvt = sb2.tile([Pn, NBn], f32, tag="lvt", name="lvt")
            nc.scalar.activation(lvt[:], vart[:], AF.Ln, bias=c_1e5[:])
            rstdt = sb2.tile([Pn, NBn], f32, tag="rstdt", name="rstdt")
            nc.scalar.activation(rstdt[:], lvt[:], AF.Exp, scale=-0.5)
            Bt = sb2.tile([Pn, NBn], f32, tag="Bt", name="Bt")
            nc.vector.tensor_scalar(out=Bt[:], in0=mu_t[:], scalar1=-1.0,
                                    scalar2=None, op0=OP.mult)
            nc.vector.tensor_mul(Bt[:], Bt[:], rstdt[:])
            for st in range(NBn):
                if p["ln_identity"]:
                    u = sb2.tile([Pn, Dn], f32, tag="u", name="u")
                    nc.scalar.activation(u[:], h2T_sb[:, st, :], AF.Relu,
                                         scale=rstdt[:, st:st + 1],
                                         bias=Bt[:, st:st + 1])
                    nc.vector.tensor_add(h_sb[:, st, :], u[:], h_sb[:, st, :])
                else:
                    u = sb2.tile([Pn, Dn], f32, tag="u", name="u")
                    nc.scalar.activation(u[:], h2T_sb[:, st, :], AF.Identity,
                                         scale=rstdt[:, st:st + 1],
                                         bias=Bt[:, st:st + 1])
                    nc.vector.tensor_mul(u[:], u[:], lng_bc[l][:])
                    nc.vector.tensor_add(u[:], u[:], lnb_bc[l][:])
                    nc.vector.tensor_scalar(out=u[:], in0=u[:], scalar1=0.0,
                                            scalar2=None, op0=OP.max)
                    nc.vector.tensor_add(h_sb[:, st, :], u[:], h_sb[:, st, :])

        # ---- out-projection: out = h @ out_w (+ out_b) ----
        out_sb = sb.tile([Pn, NBn, COUT], f32, tag="out_sb")
        for b in range(NBn):
            hT_ps = pp.tile([Pn, Dn], f32, tag="psm", name="hT_ps")
            nc.tensor.transpose(hT_ps[:], h_sb[:, b, :], ident[:])
            hT = sb2.tile([Pn, Dn], f32, tag="hT", name="hT")
            nc.scalar.activation(hT[:], hT_ps[:], AF.Copy)
            o_ps = pp.tile([Pn, COUT], f32, tag="psm", name="o_ps")
            nc.tensor.matmul(o_ps[:], lhsT=hT[:], rhs=out_w_sb[:],
                             start=True, stop=True)
            nc.scalar.activation(out_sb[:, b, :], o_ps[:], AF.Copy)
            if not p["out_b_zero"]:
                nc.vector.tensor_add(out_sb[:, b, :], out_sb[:, b, :],
                                     outb_bc[:])
        nc.sync.dma_start(
            out=out_d.rearrange("(b p) f -> p b f", p=Pn),
            in_=out_sb[:])

    _pin_act_tables()
    _fix_swdge_bump_queues(nc)
    nc.compile()
    return nc


def _fix_swdge_bump_queues(nc):
    """Tile emits the DMASW sem-bump (InstIncSwdgeSem) for prepare_only
    SWDGE preps with queue_num=0 regardless of the prep's queue. Our preps
    cycle queues exactly like Tile cycles DMASW lanes (j % 4), so lane i's
    bump belongs on queue i."""
    from concourse import bass_isa
    for b in nc.main_func.blocks:
        for i in b.instructions:
            if isinstance(i, bass_isa.InstIncSwdgeSem) and i._mode == "add":
                names = i._sem_names
                if names and names[0].startswith("DMASW"):
                    lane = int(names[0][5:].split("_")[0])
                    i.queue_num = lane % 4


def _pin_act_tables():
    """Force all activation funcs onto natural_log_exp_and_others so the
    kernel needs exactly one ACT table load (Exp/Ln/Copy/Relu/Identity are
    all members). Default placement ping-pongs exp_and_others <->
    natural_log, costing ~1.3us per switch."""
    import concourse.bacc as bacc_mod
    import concourse.hw_specs as hw_specs_mod
    if getattr(bacc_mod, "_act_tables_pinned", False):
        return
    orig = hw_specs_mod.get_activation_tables

    def pinned(arch):
        t = orig(arch)
        keep = "natural_log_exp_and_others"
        return {name: (fns if name == keep else set())
                for name, fns in t.items()}

    bacc_mod.get_activation_tables = pinned
    bacc_mod._act_tables_pinned = True


# ---------------------------------------------------------------------------
# Host-side data prep
# ---------------------------------------------------------------------------

def prep_edges(edge_index, p):
    """Group edges by (dst core, dst block), split each block's edges into a
    lo segment (src row < LOSPLIT) and a hi segment, pad each segment to a
    multiple of 128 slots (pad gather idx 0, pad one-hot col 200), and build
    the int16 wrapped gather-index tensor per group of GRP blocks."""
    Wn, Pn, NBn, GRP = p["W"], p["P"], p["NB"], p["GRP"]
    SHR, SHn, LOSPLIT = p["SH_REAL"], p["SH"], p["LOSPLIT"]
    NG = math.ceil(NBn / GRP)
    src = edge_index[0].astype(np.int64)
    dst = edge_index[1].astype(np.int64)
    src_pad = (src // SHR) * SHn + (src % SHR)
    core = dst // SHR
    dstl = dst % SHR
    blk = dstl // Pn
    col = (dstl % Pn).astype(np.float32)
    hi = (src_pad >= LOSPLIT).astype(np.int64)
    # order edges by (core, block, hi) so each segment is contiguous
    key = (core * NBn + blk) * 2 + hi
    order = np.lexsort((src_pad, key))
    counts = np.bincount(key, minlength=Wn * NBn * 2)
    cl = counts[0::2].reshape(Wn, NBn)
    ch = counts[1::2].reshape(Wn, NBn)
    CAPL = max(1, int(math.ceil(cl.max() / Pn)))
    CAPH = int(math.ceil(ch.max() / Pn))
    CAP = CAPL + CAPH
    starts = np.zeros(Wn * NBn * 2, np.int64)
    starts[1:] = np.cumsum(counts)[:-1]
    ne = len(src)
    ko = key[order]
    pos = np.arange(ne) - starts[ko]          # position within segment
    seg_cap = np.where(np.arange(Wn * NBn * 2) % 2 == 0, CAPL, CAPH) * Pn
    # slot index within the (core, block) padded layout:
    #  lo edges:   slot = pos           (< CAPL*128)
    #  hi edges:   slot = CAPL*128 + pos
    slot = pos + (ko % 2) * CAPL * Pn
    cb = ko // 2                               # core*NB + blk
    # gather index value: row within its table (lo: src_pad, hi: -LOSPLIT)
    gidx = (src_pad[order] - hi[order] * LOSPLIT).astype(np.int16)
    idxs = np.zeros((Wn * NBn, CAP * Pn), np.int16)
    colb = np.full((Wn * NBn, CAP * Pn), 200.0, np.float32)
    idxs[cb, slot] = gidx
    colb[cb, slot] = col[order]
    # one-hot column tensor: [W, NB, 128, CAP]  (chunk-transposed)
    colb = np.ascontiguousarray(
        colb.reshape(Wn, NBn, CAP, Pn).transpose(0, 1, 3, 2)).astype(
            np.float16)
    # gather index tensor per group: [W, NG, 128, GRP*CAP*8]
    NBpad = NG * GRP
    idxs_pad = np.zeros((Wn, NBpad, CAP * Pn), np.int16)
    idxs_pad[:, :NBn] = idxs.reshape(Wn, NBn, CAP * Pn)
    idxs_pad = idxs_pad.reshape(Wn, NG, GRP, CAP * Pn)
    lo_part = idxs_pad[:, :, :, :CAPL * Pn].reshape(Wn, NG, GRP * CAPL * Pn)
    hi_part = idxs_pad[:, :, :, CAPL * Pn:].reshape(Wn, NG, GRP * CAPH * Pn)
    flat = np.concatenate([lo_part, hi_part], axis=2)  # [W, NG, GRP*CAP*128]
    nflat = flat.shape[2]
    wrapped = np.zeros((Wn, NG, 16, nflat // 16), np.int16)
    ii = np.arange(nflat)
    wrapped[:, :, ii % 16, ii // 16] = flat
    idx16 = np.ascontiguousarray(
        np.tile(wrapped, (1, 1, 8, 1)))  # [W, NG, 128, GRP*CAP*8]
    return idx16, colb, CAPL, CAPH


def prep_in_maps(inputs, p, idx16, colb):
    Wn, Pn = p["W"], p["P"]
    SHR, SHn = p["SH_REAL"], p["SH"]
    x = np.asarray(inputs["x"], np.float32)
    in_maps = []
    for k in range(Wn):
        xs = np.zeros((SHn, x.shape[1]), np.float32)
        xs[:SHR] = x[k * SHR:(k + 1) * SHR]
        m = {
            "x_fm": np.ascontiguousarray(xs.T),
            "idx16": idx16[k],
            "dst_col": colb[k],
            "in_w": np.asarray(inputs["in_w"], np.float32),
            "w1": np.asarray(inputs["w1"], np.float32),
            "w2": np.asarray(inputs["w2"], np.float32),
            "bn_g": np.asarray(inputs["bn_g"], np.float32),
            "bn_b": np.asarray(inputs["bn_b"], np.float32),
            "out_w": np.asarray(inputs["out_w"], np.float32),
        }
        if not p["b2_zero"]:
            m["b2"] = np.asarray(inputs["b2"], np.float32)
        if not p["t_one"]:
            m["t"] = np.asarray(inputs["t"], np.float32)
        if not p["in_b_zero"]:
            m["in_b"] = np.asarray(inputs["in_b"], np.float32)
        if not p["out_b_zero"]:
            m["out_b"] = np.asarray(inputs["out_b"], np.float32)
        if not p["ln_identity"]:
            m["ln_g"] = np.asarray(inputs["ln_g"], np.float32)
            m["ln_b"] = np.asarray(inputs["ln_b"], np.float32)
        in_maps.append(m)
    return in_maps


def detect_fastpath(inputs, p):
    p["t_one"] = bool(np.all(np.asarray(inputs["t"]) == 1.0))
    p["in_b_zero"] = bool(np.all(np.asarray(inputs["in_b"]) == 0.0))
    p["out_b_zero"] = bool(np.all(np.asarray(inputs["out_b"]) == 0.0))
    p["b2_zero"] = bool(np.all(np.asarray(inputs["b2"]) == 0.0))
    p["ln_identity"] = bool(
        np.all(np.asarray(inputs["ln_g"]) == 1.0)
        and np.all(np.asarray(inputs["ln_b"]) == 0.0))
    # b1 is skipped unconditionally: it cancels exactly in BatchNorm.
    return p


_PROGRAM_CACHE = {}


def _get_program(p):
    key = (p["CAPL"], p["CAPH"], p["t_one"], p["in_b_zero"],
           p["out_b_zero"], p["b2_zero"], p["ln_identity"])
    if key not in _PROGRAM_CACHE:
        _PROGRAM_CACHE[key] = build_program(p)
    return _PROGRAM_CACHE[key]


def _ensure_ntff_hook():
    """Register the axon NTFF profiling hook (the image's antenv package
    lacks axon_hooks; inject an equivalent module)."""
    import types
    if "antenv.axon_hooks" in sys.modules:
        return
    sys.path.insert(0, "/root/.axon_site")
    from trn_agent_boot.trn_boot import _ntff_profile_via_ctypes
    hook = _ntff_profile_via_ctypes("/opt/axon/libaxon_pjrt.so")
    mod = types.ModuleType("antenv.axon_hooks")
    mod._hook = hook
    mod.set_axon_ntff_profile_hook = lambda h: setattr(mod, "_hook", h)
    mod.get_axon_ntff_profile_hook = lambda: mod._hook
    sys.modules["antenv.axon_hooks"] = mod


def run(inputs, trace=False, trace_cores=None):
    from concourse.bass_utils import run_bass_kernel_spmd
    if trace:
        _ensure_ntff_hook()
    p = default_params()
    detect_fastpath(inputs, p)
    idx16, colb, CAPL, CAPH = prep_edges(
        np.asarray(inputs["edge_index"]), p)
    p["CAPL"], p["CAPH"] = CAPL, CAPH
    nc = _get_program(p)
    in_maps = prep_in_maps(inputs, p, idx16, colb)
    kwargs = {}
    if trace:
        kwargs = dict(trace=True,
                      trace_cores=trace_cores or [0])
    bkr = run_bass_kernel_spmd(nc, in_maps, core_ids=list(range(p["W"])),
                               **kwargs)
    outs = []
    for k in range(p["W"]):
        outs.append(np.asarray(bkr.results[k]["out"])[:p["SH_REAL"]])
    full = np.concatenate(outs, axis=0).astype(np.float32)
    return full, bkr


def kernel(**inputs):
    full, _ = run(inputs, trace=False)
    return full

